# revision 1
# baseline (speedup 1.0000x reference)
"""Trainium2 Bass kernel for BertAttention (B=16, S=1024, H=768, 12 heads).

Data-parallel over batch across 8 NeuronCores (2 batch rows per core).

Host side (in kernel()): weights are pre-transposed to lhsT layout,
pre-scaled by 16 and quantized to fp8e4; x is pre-transposed/quantized
likewise.  hs stays f32 for the residual + layernorm path.

Per-core device kernel:
  - Q/K/V projections as fp8e4 DoubleRow matmuls (0.5 cyc/row, 256-deep
    contraction per matmul).
  - scores as fp8e4 DoubleRow matmuls with Q/K in a feature-paired
    [32, 2, tok] SBUF layout produced by an SBUF->SBUF DMA rearrange
    (4 half-heads per 128 partitions, 3 head-pairs along the free axis).
  - exp on ACT engine with the 1/(8*256) scale folded in, writing fp8
    probs directly; the additive mask is applied exactly as a
    multiplicative exp(mask) folded into the V rows and denominator.
  - softmax denominator folded INTO the PV matmul: lhsT = [V_h | em] for
    even heads and [em | V_h] for odd heads, so denominators accumulate on
    the other 64 PSUM partitions for free.  Division via DVE reciprocal +
    multiply with mixed-partition-offset operands (PSUM in0 + SBUF in1).
  - output projection in bf16 + residual add + LayerNorm
    (bn_stats/bn_aggr, Sqrt batched per t8 pair, gamma in bf16).
  - schedule keeps the ACT engine (the bottleneck at ~216us busy)
    saturated: PE emits scores one k-chunk ahead of PV, batch row b1's
    projections ride in b0's attention slack, and output tiles interleave
    into the following attention loop.

Workaround: this container's walrus accepts only ONE sync wait per
instruction; a post-pass splits multi-wait instructions into single-wait
NOPs.
"""

import numpy as np

import concourse.bass as bass
import concourse.mybir as mybir
import concourse.tile as tile

P = 128
H = 768
NH = 12
HD = 64
S = 1024
B = 16
NCORES = 8
BPC = B // NCORES  # 2
IO_T = H // P      # 6
KO_T = S // P      # 8
HP = NH // 2       # 6 head pairs (one per 128-feature block)
WSCALE = 16.0
EXP_SCALE = 1.0 / (8.0 * WSCALE * WSCALE)  # 1/sqrt(64) / (16*16)
LN_EPS = 1e-12

F32 = mybir.dt.float32
BF16 = mybir.dt.bfloat16
FP8 = mybir.dt.float8e4
AF = mybir.ActivationFunctionType
OP = mybir.AluOpType
PM = mybir.MatmulPerfMode


def _split_multi_waits(nc):
    """walrus here rejects >1 sync wait per instruction; hoist extras into
    single-wait NOPs on the same engine immediately before."""
    n = 0
    for blk in nc.m.functions[0].blocks:
        insts = blk.instructions
        new = []
        changed = False
        for inst in insts:
            si = inst.sync_info
            waits = list(si.on_wait) if si and si.on_wait else []
            if len(waits) > 1:
                changed = True
                for k, w in enumerate(waits[:-1]):
                    n += 1
                    new.append(
                        mybir.InstNoOp(
                            name=f"ws-{blk.name}-{inst.name}-{k}",
                            engine=inst.engine,
                            sync_info=mybir.SyncInfo(on_wait=[w], on_update=[]),
                        )
                    )
                inst.sync_info = mybir.SyncInfo(
                    on_wait=[waits[-1]], on_update=list(si.on_update)
                )
            new.append(inst)
        if changed:
            blk.instructions = new
    return n


def _bcast_ap(ap, parts=P):
    return bass.AP(tensor=ap.tensor, offset=ap.offset, ap=[[0, parts]] + list(ap.ap))


def build_bass(skip_gb=False):
    nc = bass.Bass()

    # weights / x arrive pre-transposed, pre-scaled (x16) and pre-quantized
    # to fp8e4 from the host; hs stays f32 for the residual + layernorm path.
    hs = nc.declare_dram_parameter("hs", [BPC, S, H], F32, isOutput=False)
    x8d = nc.declare_dram_parameter("x8d", [BPC, P, IO_T, S], FP8, isOutput=False)
    msk = nc.declare_dram_parameter("msk", [BPC, S], F32, isOutput=False)
    w8q = nc.declare_dram_parameter("w8q", [P, IO_T, H], FP8, isOutput=False)
    w8k = nc.declare_dram_parameter("w8k", [P, IO_T, H], FP8, isOutput=False)
    w8v = nc.declare_dram_parameter("w8v", [P, IO_T, H], FP8, isOutput=False)
    wTo = nc.declare_dram_parameter("wTo", [P, IO_T, H], BF16, isOutput=False)
    qb16d = nc.declare_dram_parameter("qb16d", [P, IO_T], F32, isOutput=False)
    kb16d = nc.declare_dram_parameter("kb16d", [P, IO_T], F32, isOutput=False)
    vb16d = nc.declare_dram_parameter("vb16d", [H], BF16, isOutput=False)
    obd = nc.declare_dram_parameter("obd", [H], BF16, isOutput=False)
    gamma = nc.declare_dram_parameter("gamma", [H], F32, isOutput=False)
    beta = nc.declare_dram_parameter("beta", [H], F32, isOutput=False)
    out = nc.declare_dram_parameter("out", [BPC, S, H], F32, isOutput=True)

    from contextlib import ExitStack

    with tile.TileContext(nc) as tc:
        with ExitStack() as ctx:
            _build_tile(
                ctx, tc, nc, hs, x8d, msk, w8q, w8k, w8v, wTo,
                qb16d, kb16d, vb16d, obd, gamma, beta, out, skip_gb
            )

    _split_multi_waits(nc)
    return nc


def _build_tile(ctx, tc, nc, hs, x8d, msk, w8q, w8k, w8v, wTo,
                qb16d, kb16d, vb16d, obd, gamma, beta, out, skip_gb=False):
    dram = ctx.enter_context(tc.tile_pool(name="dram", bufs=1, space="DRAM"))
    consts = ctx.enter_context(tc.tile_pool(name="consts", bufs=1))
    perb = ctx.enter_context(tc.tile_pool(name="perb", bufs=2))
    x8_pool = ctx.enter_context(tc.tile_pool(name="x8", bufs=2))
    pre_pool = ctx.enter_context(tc.tile_pool(name="pre", bufs=2))
    qk8_pool = ctx.enter_context(tc.tile_pool(name="qk8", bufs=4))
    ve_pool = ctx.enter_context(tc.tile_pool(name="ve", bufs=2))
    pt_pool = ctx.enter_context(tc.tile_pool(name="pt", bufs=2))
    rcp_pool = ctx.enter_context(tc.tile_pool(name="rcp", bufs=2))
    ctxT_pool = ctx.enter_context(tc.tile_pool(name="ctxT", bufs=2))
    xres_pool = ctx.enter_context(tc.tile_pool(name="xres", bufs=2))
    s_pool = ctx.enter_context(tc.tile_pool(name="s", bufs=4))
    n_pool = ctx.enter_context(tc.tile_pool(name="n", bufs=2))
    o_pool = ctx.enter_context(tc.tile_pool(name="o", bufs=2))
    ln_pool = ctx.enter_context(tc.tile_pool(name="ln", bufs=4))

    ps_proj = ctx.enter_context(tc.tile_pool(name="psp", bufs=2, space="PSUM"))
    ps_sc = ctx.enter_context(tc.tile_pool(name="pssc", bufs=2, space="PSUM"))
    ps_pv = ctx.enter_context(tc.tile_pool(name="pspv", bufs=1, space="PSUM"))

    # ---------------- constants / weight staging --------------------------
    w8 = {}
    x8s = []
    for b in range(BPC):
        x8s.append(x8_pool.tile([P, IO_T, S], FP8, tag="x8", name=f"x8_{b}"))
    for name in ("q", "k", "v"):
        w8[name] = consts.tile([P, IO_T, H], FP8, tag=f"w8_{name}", name=f"w8_{name}")
    wT_o = consts.tile([P, IO_T, H], BF16, tag="wT_o", name="wT_o")
    # critical-path loads first; the rest are emitted after emit_qk_proj(0)
    nc.sync.dma_start(out=x8s[0], in_=x8d[:, :, :, :][0])
    nc.scalar.dma_start(out=w8["q"], in_=w8q[:, :, :])
    nc.scalar.dma_start(out=w8["k"], in_=w8k[:, :, :])

    def stage_rest():
        nc.sync.dma_start(out=w8["v"], in_=w8v[:, :, :])
        nc.sync.dma_start(out=x8s[1], in_=x8d[:, :, :, :][1])
        nc.sync.dma_start(out=wT_o, in_=wTo[:, :, :])

    qb16 = consts.tile([P, IO_T], F32, tag="qb16")
    nc.sync.dma_start(out=qb16, in_=qb16d[:, :])
    kb16 = consts.tile([P, IO_T], F32, tag="kb16")
    nc.sync.dma_start(out=kb16, in_=kb16d[:, :])
    vb16_row = consts.tile([1, H], BF16, tag="vb16_row")
    nc.sync.dma_start(out=vb16_row, in_=vb16d[:][None, :])
    ob_row = consts.tile([1, H], BF16, tag="ob_row")
    nc.sync.dma_start(out=ob_row, in_=obd[:][None, :])

    gamma_bc = consts.tile([P, H], BF16, tag="gamma_bc")
    nc.gpsimd.dma_start(out=gamma_bc, in_=_bcast_ap(gamma[:]))
    beta_bc = consts.tile([P, H], F32, tag="beta_bc")
    nc.gpsimd.dma_start(out=beta_bc, in_=_bcast_ap(beta[:]))

    eps_sb = consts.tile([P, 1], F32, tag="eps")
    nc.vector.memset(eps_sb, LN_EPS)
    ones_row = consts.tile([1, P], BF16, tag="ones_row")
    nc.vector.memset(ones_row, 1.0)
    ones16 = consts.tile([P, 6, HD], BF16, tag="ones16")
    nc.vector.memset(ones16, WSCALE)

    # ---------------- per-b state ----------------------------------------
    qk_tiles = {}
    em_sbs = [None] * BPC
    Q8s = [None] * BPC
    K8s = [None] * BPC
    VEs = [None] * BPC
    ctxTs = [None] * BPC

    def emit_mask(b):
        mask_sb = perb.tile([P, KO_T], F32, tag="mask")
        nc.sync.dma_start(out=mask_sb, in_=msk[:, :][b].rearrange("(o p) -> p o", p=P))
        em_sbs[b] = perb.tile([P, KO_T], F32, tag="em", name=f"em_{b}")
        nc.scalar.activation(out=em_sbs[b], in_=mask_sb, func=AF.Exp)

    def emit_qk_proj(b, names=("q", "k"), use_act=False, gs=(0, 1)):
        """Q/K projections (fp8 DR) -> fp8 pre tiles -> DMA pair-rearrange.

        Emitted g-outer (3-jo group), tensor-inner, so the first head-pairs
        of BOTH Q and K are ready before later groups.  use_act routes the
        PSUM->fp8+bias copies to the ACT engine (idle at startup)."""
        x8 = x8s[b]
        cfg = {"q": (qb16, Q8s), "k": (kb16, K8s)}
        tiles = {}
        for wname in names:
            if (b, wname) not in qk_tiles:
                qk_tiles[(b, wname)] = (
                    pre_pool.tile([P, IO_T, S], FP8, tag="pre", name=f"{wname}pre_{b}"),
                    qk8_pool.tile([P, 2, 3, S], FP8, tag="qk8", name=f"{wname}8_{b}"),
                )
            tiles[wname] = qk_tiles[(b, wname)]
            cfg[wname][1][b] = tiles[wname][1]
        for g in gs:
            for wname in names:
                bias, _ = cfg[wname]
                pre, paired = tiles[wname]
                for jo in range(3 * g, 3 * g + 3):
                    for tt in range(2):
                        ps = ps_proj.tile([P, 512], F32, tag="proj")
                        for i2 in range(3):
                            lhsT = w8[wname][:, 2 * i2 : 2 * i2 + 2, jo * P : (jo + 1) * P]
                            for nq in range(2):
                                nc.tensor.matmul(
                                    ps[:, nq * 256 : (nq + 1) * 256],
                                    lhsT=lhsT,
                                    rhs=x8[:, 2 * i2 : 2 * i2 + 2,
                                          tt * 512 + nq * 256 : tt * 512 + (nq + 1) * 256],
                                    start=(i2 == 0),
                                    stop=(i2 == 2),
                                    perf_mode=PM.DoubleRow,
                                )
                        if use_act:
                            nc.scalar.activation(
                                out=pre[:, jo, tt * 512 : (tt + 1) * 512],
                                in_=ps,
                                func=AF.Identity,
                                bias=bias[:, jo : jo + 1],
                            )
                        else:
                            nc.vector.tensor_scalar_add(
                                out=pre[:, jo, tt * 512 : (tt + 1) * 512],
                                in0=ps,
                                scalar1=bias[:, jo : jo + 1],
                            )
                # pair-rearrange: head-pair hp -> (g = hp//3, s = hp%3);
                # half-head (hp, A) at partitions 64g..64g+32, (hp, B) at +32.
                # paired[p, i, s, n]: feature 2(p%32)+i of that half-head.
                for half in range(2):
                    src = pre[64 * half : 64 * half + 64, 3 * g : 3 * g + 3, :].rearrange(
                        "(p i) jo n -> p i jo n", i=2
                    )
                    pb = 64 * g + 32 * half
                    for i in range(2):
                        nc.sync.dma_start(
                            out=paired[pb : pb + 32, i, :, :], in_=src[:, i, :, :]
                        )

    def emit_v_proj(b, t8s=None):
        """V projection (fp8 DR); write VE = per-head [V|em] / [em|V] fp8."""
        x8 = x8s[b]
        em_sb = em_sbs[b]
        if VEs[b] is None:
            VEs[b] = ve_pool.tile([P, KO_T, NH, P], FP8, tag="VE", name=f"VE_{b}")
        VE = VEs[b]
        for t8 in (t8s if t8s is not None else range(KO_T)):
            # em columns: even heads cols 64:128, odd heads cols 0:64
            ve_all = VE[:, t8, :, :].rearrange("p (hh two) d -> p hh two d", two=2)
            nc.vector.tensor_scalar_mul(
                out=ve_all[:, :, 0, HD:P],
                in0=ones16,
                scalar1=em_sb[:, t8 : t8 + 1],
            )
            nc.vector.tensor_scalar_mul(
                out=ve_all[:, :, 1, 0:HD],
                in0=ones16,
                scalar1=em_sb[:, t8 : t8 + 1],
            )
            for jh in range(2):
                ps = ps_proj.tile([P, 512], F32, tag="proj")
                for i2 in range(3):
                    lhsT = x8[:, 2 * i2 : 2 * i2 + 2, t8 * P : (t8 + 1) * P]
                    for nv in range(2):
                        nc.tensor.matmul(
                            ps[:, nv * 192 : (nv + 1) * 192],
                            lhsT=lhsT,
                            rhs=w8["v"][:, 2 * i2 : 2 * i2 + 2,
                                        jh * 384 + nv * 192 : jh * 384 + (nv + 1) * 192],
                            start=(i2 == 0),
                            stop=False,
                            perf_mode=PM.DoubleRow,
                        )
                nc.tensor.matmul(
                    ps[:, 0:384],
                    lhsT=ones_row,
                    rhs=vb16_row[:, jh * 384 : (jh + 1) * 384],
                    start=False,
                    stop=True,
                )
                # heads 6jh..6jh+5 live in psum cols (h-6jh)*64;
                # even heads -> V cols 0:64, odd heads -> V cols 64:128
                ps_v = ps[:, 0:384].rearrange(
                    "p (hh two d) -> p hh two d", two=2, d=HD
                )
                ve_jh = VE[:, t8, 6 * jh : 6 * jh + 6, :].rearrange(
                    "p (hh two) d -> p hh two d", two=2
                )
                nc.vector.tensor_scalar_mul(
                    out=ve_jh[:, :, 0, 0:HD],
                    in0=ps_v[:, :, 0, :],
                    scalar1=em_sb[:, t8 : t8 + 1],
                )
                nc.vector.tensor_scalar_mul(
                    out=ve_jh[:, :, 1, HD:P],
                    in0=ps_v[:, :, 1, :],
                    scalar1=em_sb[:, t8 : t8 + 1],
                )

    def emit_attn(b, hp, qt, cbs=None, direct=False):
        """scores (fp8 DR) -> exp -> PV(+denominator) -> divide, one q-chunk."""
        Q8, K8 = Q8s[b], K8s[b]
        VE = VEs[b]
        if ctxTs[b] is None:
            ctxTs[b] = ctxT_pool.tile([P, HP, S], BF16, tag="ctxT", name=f"ctxT_{b}")
        ctxT = ctxTs[b]
        qsl0 = qt * 512
        pt = pt_pool.tile([P, 2, KO_T, 512], FP8, tag="pt")
        ctxpA = ps_pv.tile([P, 512], F32, tag="pvA")
        ctxpB = ps_pv.tile([P, 512], F32, tag="pvB")
        g, sslot = hp // 3, hp % 3

        def emit_scores(kc):
            for dst, pbase in ((0, 64 * g), (1, 64 * g + 32)):
                sc = ps_sc.tile([P, 2, 512], F32, tag="sc")
                for k2 in range(2):
                    ko = kc * 2 + k2
                    if direct:
                        # read straight from the un-paired pre tiles (non-DR
                        # fp8, 2x PE cost) -- used only for the first chunk to
                        # skip the pair-rearrange DMA on the critical path.
                        preq = qk_tiles[(b, "q")][0]
                        prek = qk_tiles[(b, "k")][0]
                        db = 64 * dst
                        for nq in range(2):
                            nc.tensor.matmul(
                                sc[:, k2, nq * 256 : (nq + 1) * 256],
                                lhsT=prek[db : db + 64, hp, ko * P : (ko + 1) * P],
                                rhs=preq[db : db + 64, hp,
                                         qsl0 + nq * 256 : qsl0 + (nq + 1) * 256],
                                start=True,
                                stop=True,
                                tile_position=(db, 0),
                            )
                        continue
                    lhsT = K8[pbase : pbase + 32, :, sslot, ko * P : (ko + 1) * P]
                    for nq in range(2):
                        nc.tensor.matmul(
                            sc[:, k2, nq * 256 : (nq + 1) * 256],
                            lhsT=lhsT,
                            rhs=Q8[pbase : pbase + 32, :, sslot,
                                   qsl0 + nq * 256 : qsl0 + (nq + 1) * 256],
                            start=True,
                            stop=True,
                            perf_mode=PM.DoubleRow,
                            tile_position=(pbase, 0),
                        )
                nc.scalar.activation(
                    out=pt[:, dst, kc * 2 : kc * 2 + 2, :],
                    in_=sc,
                    func=AF.Exp,
                    scale=EXP_SCALE,
                )

        def emit_pv(kc):
            for k2 in range(2):
                ko = kc * 2 + k2
                nc.tensor.matmul(
                    ctxpA,
                    lhsT=VE[:, ko, 2 * hp, :],
                    rhs=pt[:, 0, ko, :],
                    start=(ko == 0),
                    stop=(ko == KO_T - 1),
                )
                nc.tensor.matmul(
                    ctxpB,
                    lhsT=VE[:, ko, 2 * hp + 1, :],
                    rhs=pt[:, 1, ko, :],
                    start=(ko == 0),
                    stop=(ko == KO_T - 1),
                )

        # PE order: sc0 sc1 pv0 sc2 pv1 sc3 pv2 pv3 -- keeps the exp stream
        # fed one chunk ahead so ACT never waits on PV matmuls.  cbs inject
        # foreign PE work (output-projection halves) into the slack.
        for kc in range(KO_T // 2):
            emit_scores(kc)
            if cbs and kc in cbs:
                cbs[kc]()
            if kc >= 1:
                emit_pv(kc - 1)
        emit_pv(KO_T // 2 - 1)
        # ctxpA: rows 0:64 = 16*ctx_A, rows 64:128 = 16*den_A
        # ctxpB: rows 0:64 = 16*den_B, rows 64:128 = 16*ctx_B
        rcpT = rcp_pool.tile([P, 512], F32, tag="rcpT")
        nc.vector.reciprocal(out=rcpT[HD:P, :], in_=ctxpA[HD:P, :])
        nc.vector.reciprocal(out=rcpT[0:HD, :], in_=ctxpB[0:HD, :])
        nc.vector.tensor_tensor(
            out=ctxT[0:HD, hp, qsl0 : qsl0 + 512],
            in0=ctxpA[0:HD, :],
            in1=rcpT[HD:P, :],
            op=OP.mult,
        )
        nc.vector.tensor_tensor(
            out=ctxT[HD:P, hp, qsl0 : qsl0 + 512],
            in0=ctxpB[HD:P, :],
            in1=rcpT[0:HD, :],
            op=OP.mult,
        )

    # ---- output projection + residual + layernorm ------------------------
    mv_alls = [None] * BPC
    rstds = [None] * BPC
    s_tiless = [[], []]

    def emit_out_jh(b, t8, jh, state):
        ctxT = ctxTs[b]
        if jh == 0:
            state["xres"] = xres_pool.tile([P, H], F32, tag="xres", name=f"xres_{b}_{t8}")
            nc.sync.dma_start(
                out=state["xres"], in_=hs[b, t8 * P : (t8 + 1) * P, :]
            )
            state["s_t"] = s_pool.tile([P, H], F32, tag="s", name=f"s_{b}_{t8}")
        xres = state["xres"]
        s_t = state["s_t"]
        ps = ps_proj.tile([P, 512], F32, tag="proj")
        for io in range(IO_T):
            nc.tensor.matmul(
                ps[:, 0:384],
                lhsT=ctxT[:, io, t8 * P : (t8 + 1) * P],
                rhs=wT_o[:, io, jh * 384 : (jh + 1) * 384],
                start=(io == 0),
                stop=False,
            )
        nc.tensor.matmul(
            ps[:, 0:384],
            lhsT=ones_row,
            rhs=ob_row[:, jh * 384 : (jh + 1) * 384],
            start=False,
            stop=True,
        )
        nc.vector.tensor_tensor(
            out=s_t[:, jh * 384 : (jh + 1) * 384],
            in0=ps[:, 0:384],
            in1=xres[:, jh * 384 : (jh + 1) * 384],
            op=OP.add,
        )

    def emit_out(b, t8, pool_gb=False, state=None, rbatch=2):
        if mv_alls[b] is None:
            mv_alls[b] = ln_pool.tile([P, KO_T, 2], F32, tag="mv", name=f"mv_{b}")
            rstds[b] = ln_pool.tile([P, KO_T], F32, tag="rstd", name=f"rstd_{b}")
        mv_all = mv_alls[b]
        rstd = rstds[b]
        s_tiles = s_tiless[b]

        if state is None:
            state = {}
            emit_out_jh(b, t8, 0, state)
            emit_out_jh(b, t8, 1, state)
        s_t = state["s_t"]
        stats = ln_pool.tile([P, 3, 6], F32, tag="stats")
        for sg in range(3):
            nc.vector.bn_stats(
                out=stats[:, sg, :], in_=s_t[:, sg * 256 : (sg + 1) * 256]
            )
        nc.vector.bn_aggr(out=mv_all[:, t8, :], in_=stats)
        s_tiles.append(s_t)

        if t8 % rbatch == rbatch - 1:
            h0 = t8 - (rbatch - 1)
            nc.scalar.activation(
                out=rstd[:, h0 : t8 + 1],
                in_=mv_all[:, h0 : t8 + 1, 1],
                func=AF.Sqrt,
                bias=eps_sb,
                scale=1.0,
            )
            nc.vector.reciprocal(out=rstd[:, h0 : t8 + 1], in_=rstd[:, h0 : t8 + 1])
            for u8 in range(h0, t8 + 1):
                eng = (nc.gpsimd if (pool_gb and u8 % 2 == 0 and not skip_gb)
                       else nc.vector)
                n_t = n_pool.tile([P, H], F32 if skip_gb else BF16, tag="n")
                eng.tensor_scalar(
                    out=n_t,
                    in0=s_tiles[u8],
                    scalar1=mv_all[:, u8, 0:1],
                    scalar2=rstd[:, u8 : u8 + 1],
                    op0=OP.subtract,
                    op1=OP.mult,
                )
                if skip_gb:
                    # gamma==1, beta==0: the affine step is the identity
                    nc.sync.dma_start(
                        out=out[b, u8 * P : (u8 + 1) * P, :], in_=n_t
                    )
                    continue
                g_t = o_pool.tile([P, H], BF16, tag="g")
                eng.tensor_tensor(out=g_t, in0=n_t, in1=gamma_bc, op=OP.mult)
                o_t = o_pool.tile([P, H], F32, tag="o")
                eng.tensor_tensor(out=o_t, in0=g_t, in1=beta_bc, op=OP.add)
                nc.sync.dma_start(out=out[b, u8 * P : (u8 + 1) * P, :], in_=o_t)

    # ---------------- schedule -------------------------------------------
    emit_mask(0)
    emit_mask(1)
    emit_qk_proj(0, use_act=True)
    stage_rest()
    emit_v_proj(0)

    # b0 attention, qt-major.  b1 projections slot into PE gaps; b0 output
    # tiles start as soon as all heads of a q-chunk are done.
    for hp in range(HP):
        emit_attn(0, hp, 0)
        if hp == 2:
            emit_qk_proj(1, names=("q",), gs=(0,))
        if hp == 3:
            emit_qk_proj(1, names=("q",), gs=(1,))
        if hp == 4:
            emit_qk_proj(1, names=("k",), gs=(0,))
        if hp == 5:
            emit_qk_proj(1, names=("k",), gs=(1,))
    def attn_with_out(ab, hp, qt, ob, t8, pool_gb=False):
        emit_attn(ab, hp, qt)
        emit_out(ob, t8, pool_gb=pool_gb)

    for hp in range(HP):
        if 1 <= hp <= 4:
            attn_with_out(0, hp, 1, 0, hp - 1, pool_gb=True)
        else:
            emit_attn(0, hp, 1)
        if hp == 0:
            emit_v_proj(1)
    for hp in range(HP):
        if hp <= 3:
            attn_with_out(1, hp, 0, 0, 4 + hp, pool_gb=True)
        else:
            emit_attn(1, hp, 0)
    for hp in range(HP):
        if hp <= 3:
            attn_with_out(1, hp, 1, 1, hp, pool_gb=True)
        else:
            emit_attn(1, hp, 1)
    for t8 in range(4, KO_T):
        emit_out(1, t8, rbatch=1, pool_gb=True)


_nc_cache = {}


def _get_nc(skip_gb=True):
    if _nc_cache.get(skip_gb) is None:
        _nc_cache[skip_gb] = build_bass(skip_gb)
    return _nc_cache[skip_gb]


def kernel(**inputs):
    import ml_dtypes
    from concourse.bass_utils import run_bass_kernel_spmd

    E4M3 = ml_dtypes.float8_e4m3
    BF = ml_dtypes.bfloat16

    def wt8(w):
        # w [H, H] (torch Linear weight): lhsT layout [128, IO_T, H] of 16*w^T
        wt = np.asarray(w, np.float32).T.reshape(IO_T, P, H).transpose(1, 0, 2)
        return np.ascontiguousarray((wt * WSCALE).astype(E4M3))

    hs = np.asarray(inputs["hidden_states"], np.float32)
    mask = np.asarray(inputs["attention_mask"], np.float32).reshape(B, S)
    # x^T fp8: [B, 128, IO_T, S]
    x8 = np.ascontiguousarray(
        hs.transpose(0, 2, 1).reshape(B, IO_T, P, S).transpose(0, 2, 1, 3).astype(E4M3)
    )
    shared = {
        "w8q": wt8(inputs["qw"]),
        "w8k": wt8(inputs["kw"]),
        "w8v": wt8(inputs["vw"]),
        "wTo": np.ascontiguousarray(
            np.asarray(inputs["ow"], np.float32).T.reshape(IO_T, P, H)
            .transpose(1, 0, 2).astype(BF)
        ),
        "qb16d": np.ascontiguousarray(
            (np.asarray(inputs["qb"], np.float32) * WSCALE).reshape(IO_T, P).T
        ),
        "kb16d": np.ascontiguousarray(
            (np.asarray(inputs["kb"], np.float32) * WSCALE).reshape(IO_T, P).T
        ),
        "vb16d": np.ascontiguousarray(
            (np.asarray(inputs["vb"], np.float32) * WSCALE).astype(BF)
        ),
        "obd": np.ascontiguousarray(np.asarray(inputs["ob"], np.float32).astype(BF)),
        "gamma": np.ascontiguousarray(np.asarray(inputs["gamma"], np.float32)),
        "beta": np.ascontiguousarray(np.asarray(inputs["beta"], np.float32)),
    }
    in_maps = []
    for c in range(NCORES):
        m = dict(shared)
        m["hs"] = np.ascontiguousarray(hs[c * BPC : (c + 1) * BPC])
        m["x8d"] = np.ascontiguousarray(x8[c * BPC : (c + 1) * BPC])
        m["msk"] = np.ascontiguousarray(mask[c * BPC : (c + 1) * BPC])
        in_maps.append(m)

    # A rare per-process DMA race can corrupt a core's staging buffer, which
    # surfaces as NaN/Inf.  Sticky per module load: rebuild after 2 failures.
    skip_gb = bool(
        np.all(shared["gamma"] == 1.0) and np.all(shared["beta"] == 0.0)
    )
    out = None
    for attempt in range(6):
        res = run_bass_kernel_spmd(
            _get_nc(skip_gb), in_maps, core_ids=list(range(NCORES))
        )
        out = np.concatenate([res.results[c]["out"] for c in range(NCORES)], axis=0)
        if np.isfinite(out).all():
            break
        if attempt >= 1:
            _nc_cache[skip_gb] = None
    return out



# revision 16
# speedup vs baseline: 1.0518x; 1.0518x over previous
"""Trainium2 Bass kernel for BertAttention (B=16, S=1024, H=768, 12 heads).

Data-parallel over batch across 8 NeuronCores (2 batch rows per core).

Host side (in kernel()): weights are pre-transposed to lhsT layout,
pre-scaled by 16 and quantized to fp8e4; x is pre-transposed/quantized
likewise.  hs stays f32 for the residual + layernorm path.

Per-core device kernel:
  - Q/K/V projections as fp8e4 DoubleRow matmuls (0.5 cyc/row, 256-deep
    contraction per matmul).
  - scores as fp8e4 DoubleRow matmuls with Q/K in a feature-paired
    [32, 2, tok] SBUF layout produced by an SBUF->SBUF DMA rearrange
    (4 half-heads per 128 partitions, 3 head-pairs along the free axis).
  - exp on ACT engine with the 1/(8*256) scale folded in, writing fp8
    probs directly; the additive mask is applied exactly as a
    multiplicative exp(mask) folded into the V rows and denominator.
  - softmax denominator folded INTO the PV matmul: lhsT = [V_h | em] for
    even heads and [em | V_h] for odd heads, so denominators accumulate on
    the other 64 PSUM partitions for free.  Division via DVE reciprocal +
    multiply with mixed-partition-offset operands (PSUM in0 + SBUF in1).
  - output projection in bf16 + residual add + LayerNorm
    (bn_stats/bn_aggr, Sqrt batched per t8 pair, gamma in bf16).
  - schedule keeps the ACT engine (the bottleneck at ~216us busy)
    saturated: PE emits scores one k-chunk ahead of PV, batch row b1's
    projections ride in b0's attention slack, and output tiles interleave
    into the following attention loop.

Workaround: this container's walrus accepts only ONE sync wait per
instruction; a post-pass splits multi-wait instructions into single-wait
NOPs.
"""

import numpy as np

import concourse.bass as bass
import concourse.mybir as mybir
import concourse.tile as tile

P = 128
H = 768
NH = 12
HD = 64
S = 1024
B = 16
NCORES = 8
BPC = B // NCORES  # 2
IO_T = H // P      # 6
KO_T = S // P      # 8
HP = NH // 2       # 6 head pairs (one per 128-feature block)
WSCALE = 16.0
EXP_SCALE = 1.0 / (8.0 * WSCALE * WSCALE)  # 1/sqrt(64) / (16*16)
LN_EPS = 1e-12

F32 = mybir.dt.float32
BF16 = mybir.dt.bfloat16
FP8 = mybir.dt.float8e4
AF = mybir.ActivationFunctionType
OP = mybir.AluOpType
PM = mybir.MatmulPerfMode


def _split_multi_waits(nc):
    """walrus here rejects >1 sync wait per instruction; hoist extras into
    single-wait NOPs on the same engine immediately before."""
    n = 0
    for blk in nc.m.functions[0].blocks:
        insts = blk.instructions
        new = []
        changed = False
        for inst in insts:
            si = inst.sync_info
            waits = list(si.on_wait) if si and si.on_wait else []
            if len(waits) > 1:
                changed = True
                for k, w in enumerate(waits[:-1]):
                    n += 1
                    new.append(
                        mybir.InstNoOp(
                            name=f"ws-{blk.name}-{inst.name}-{k}",
                            engine=inst.engine,
                            sync_info=mybir.SyncInfo(on_wait=[w], on_update=[]),
                        )
                    )
                inst.sync_info = mybir.SyncInfo(
                    on_wait=[waits[-1]], on_update=list(si.on_update)
                )
            new.append(inst)
        if changed:
            blk.instructions = new
    return n


def _bcast_ap(ap, parts=P):
    return bass.AP(tensor=ap.tensor, offset=ap.offset, ap=[[0, parts]] + list(ap.ap))


def build_bass(skip_gb=False):
    nc = bass.Bass()

    # weights / x arrive pre-transposed, pre-scaled (x16) and pre-quantized
    # to fp8e4 from the host; hs stays f32 for the residual + layernorm path.
    hs = nc.declare_dram_parameter("hs", [BPC, S, H], F32, isOutput=False)
    x8d = nc.declare_dram_parameter("x8d", [BPC, P, IO_T, S], FP8, isOutput=False)
    msk = nc.declare_dram_parameter("msk", [BPC, S], F32, isOutput=False)
    w8q = nc.declare_dram_parameter("w8q", [P, IO_T, H], FP8, isOutput=False)
    w8k = nc.declare_dram_parameter("w8k", [P, IO_T, H], FP8, isOutput=False)
    w8v = nc.declare_dram_parameter("w8v", [P, IO_T, H], FP8, isOutput=False)
    wTo = nc.declare_dram_parameter("wTo", [P, 2, 3, H], FP8, isOutput=False)
    qb16d = nc.declare_dram_parameter("qb16d", [P, IO_T], F32, isOutput=False)
    kb16d = nc.declare_dram_parameter("kb16d", [P, IO_T], F32, isOutput=False)
    vb16d = nc.declare_dram_parameter("vb16d", [H], BF16, isOutput=False)
    obd = nc.declare_dram_parameter("obd", [H], BF16, isOutput=False)
    gamma = nc.declare_dram_parameter("gamma", [H], F32, isOutput=False)
    beta = nc.declare_dram_parameter("beta", [H], F32, isOutput=False)
    out = nc.declare_dram_parameter("out", [BPC, S, H], F32, isOutput=True)

    from contextlib import ExitStack

    with tile.TileContext(nc) as tc:
        with ExitStack() as ctx:
            _build_tile(
                ctx, tc, nc, hs, x8d, msk, w8q, w8k, w8v, wTo,
                qb16d, kb16d, vb16d, obd, gamma, beta, out, skip_gb
            )

    _split_multi_waits(nc)
    return nc


def _build_tile(ctx, tc, nc, hs, x8d, msk, w8q, w8k, w8v, wTo,
                qb16d, kb16d, vb16d, obd, gamma, beta, out, skip_gb=False):
    dram = ctx.enter_context(tc.tile_pool(name="dram", bufs=1, space="DRAM"))
    consts = ctx.enter_context(tc.tile_pool(name="consts", bufs=1))
    perb = ctx.enter_context(tc.tile_pool(name="perb", bufs=2))
    x8_pool = ctx.enter_context(tc.tile_pool(name="x8", bufs=2))
    pre_pool = ctx.enter_context(tc.tile_pool(name="pre", bufs=2))
    qk8_pool = ctx.enter_context(tc.tile_pool(name="qk8", bufs=4))
    ve_pool = ctx.enter_context(tc.tile_pool(name="ve", bufs=2))
    pt_pool = ctx.enter_context(tc.tile_pool(name="pt", bufs=2))
    rcp_pool = ctx.enter_context(tc.tile_pool(name="rcp", bufs=2))
    ctxT_pool = ctx.enter_context(tc.tile_pool(name="ctxT", bufs=2))
    xres_pool = ctx.enter_context(tc.tile_pool(name="xres", bufs=2))
    s_pool = ctx.enter_context(tc.tile_pool(name="s", bufs=4))
    n_pool = ctx.enter_context(tc.tile_pool(name="n", bufs=2))
    o_pool = ctx.enter_context(tc.tile_pool(name="o", bufs=2))
    ln_pool = ctx.enter_context(tc.tile_pool(name="ln", bufs=4))

    ps_proj = ctx.enter_context(tc.tile_pool(name="psp", bufs=2, space="PSUM"))
    ps_sc = ctx.enter_context(tc.tile_pool(name="pssc", bufs=2, space="PSUM"))
    ps_pv = ctx.enter_context(tc.tile_pool(name="pspv", bufs=1, space="PSUM"))

    # ---------------- constants / weight staging --------------------------
    w8 = {}
    x8s = []
    for b in range(BPC):
        x8s.append(x8_pool.tile([P, IO_T, S], FP8, tag="x8", name=f"x8_{b}"))
    for name in ("q", "k", "v"):
        w8[name] = consts.tile([P, IO_T, H], FP8, tag=f"w8_{name}", name=f"w8_{name}")
    wT_o = consts.tile([P, 2, 3, H], FP8, tag="wT_o", name="wT_o")
    # critical-path loads first; the rest are emitted after emit_qk_proj(0)
    nc.sync.dma_start(out=x8s[0], in_=x8d[:, :, :, :][0])
    nc.scalar.dma_start(out=w8["q"], in_=w8q[:, :, :])
    nc.scalar.dma_start(out=w8["k"], in_=w8k[:, :, :])

    def stage_rest():
        nc.sync.dma_start(out=w8["v"], in_=w8v[:, :, :])
        nc.sync.dma_start(out=x8s[1], in_=x8d[:, :, :, :][1])
        nc.sync.dma_start(out=wT_o, in_=wTo[:, :, :, :])

    qb16 = consts.tile([P, IO_T], F32, tag="qb16")
    nc.sync.dma_start(out=qb16, in_=qb16d[:, :])
    kb16 = consts.tile([P, IO_T], F32, tag="kb16")
    nc.sync.dma_start(out=kb16, in_=kb16d[:, :])
    vb16_row = consts.tile([1, H], BF16, tag="vb16_row")
    nc.sync.dma_start(out=vb16_row, in_=vb16d[:][None, :])
    ob_row = consts.tile([1, H], BF16, tag="ob_row")
    nc.sync.dma_start(out=ob_row, in_=obd[:][None, :])

    gamma_bc = consts.tile([P, H], BF16, tag="gamma_bc")
    nc.gpsimd.dma_start(out=gamma_bc, in_=_bcast_ap(gamma[:]))
    beta_bc = consts.tile([P, H], F32, tag="beta_bc")
    nc.gpsimd.dma_start(out=beta_bc, in_=_bcast_ap(beta[:]))

    eps_sb = consts.tile([P, 1], F32, tag="eps")
    nc.vector.memset(eps_sb, LN_EPS)
    ones_row = consts.tile([1, P], BF16, tag="ones_row")
    nc.vector.memset(ones_row, 1.0)
    ones16 = consts.tile([P, 6, HD], BF16, tag="ones16")
    nc.vector.memset(ones16, WSCALE)

    # ---------------- per-b state ----------------------------------------
    qk_tiles = {}
    em_sbs = [None] * BPC
    Q8s = [None] * BPC
    K8s = [None] * BPC
    VEs = [None] * BPC
    ctxTs = [None] * BPC

    def emit_mask(b):
        mask_sb = perb.tile([P, KO_T], F32, tag="mask")
        nc.sync.dma_start(out=mask_sb, in_=msk[:, :][b].rearrange("(o p) -> p o", p=P))
        em_sbs[b] = perb.tile([P, KO_T], F32, tag="em", name=f"em_{b}")
        nc.scalar.activation(out=em_sbs[b], in_=mask_sb, func=AF.Exp)

    def emit_qk_proj(b, names=("q", "k"), use_act=False, gs=(0, 1), jos=None):
        """Q/K projections (fp8 DR) -> fp8 pre tiles -> DMA pair-rearrange.

        Emitted g-outer (3-jo group), tensor-inner, so the first head-pairs
        of BOTH Q and K are ready before later groups.  use_act routes the
        PSUM->fp8+bias copies to the ACT engine; default is DVE.  jos
        (absolute jo indices) selects jo-granular emission + rearrange so a
        single head-pair's Q/K can be staged on the startup critical path."""
        x8 = x8s[b]
        cfg = {"q": (qb16, Q8s), "k": (kb16, K8s)}
        tiles = {}
        for wname in names:
            if (b, wname) not in qk_tiles:
                qk_tiles[(b, wname)] = (
                    pre_pool.tile([P, IO_T, S], FP8, tag="pre", name=f"{wname}pre_{b}"),
                    qk8_pool.tile([P, 2, 3, S], FP8, tag="qk8", name=f"{wname}8_{b}"),
                )
            tiles[wname] = qk_tiles[(b, wname)]
            cfg[wname][1][b] = tiles[wname][1]

        def emit_jo(wname, jo):
            bias, _ = cfg[wname]
            pre, paired = tiles[wname]
            for tt in range(2):
                ps = ps_proj.tile([P, 512], F32, tag="proj")
                for i2 in range(3):
                    lhsT = w8[wname][:, 2 * i2 : 2 * i2 + 2, jo * P : (jo + 1) * P]
                    nc.tensor.matmul(
                        ps,
                        lhsT=lhsT,
                        rhs=x8[:, 2 * i2 : 2 * i2 + 2, tt * 512 : (tt + 1) * 512],
                        start=(i2 == 0),
                        stop=(i2 == 2),
                        perf_mode=PM.DoubleRow,
                    )
                if use_act:
                    nc.scalar.activation(
                        out=pre[:, jo, tt * 512 : (tt + 1) * 512],
                        in_=ps,
                        func=AF.Identity,
                        bias=bias[:, jo : jo + 1],
                    )
                else:
                    nc.vector.tensor_scalar_add(
                        out=pre[:, jo, tt * 512 : (tt + 1) * 512],
                        in0=ps,
                        scalar1=bias[:, jo : jo + 1],
                    )

        def rearrange(wname, g, jo_lo, jo_hi):
            # pair-rearrange: head-pair hp -> (g = hp//3, s = hp%3);
            # half-head (hp, A) at partitions 64g..64g+32, (hp, B) at +32.
            # paired[p, i, s, n]: feature 2(p%32)+i of that half-head.
            pre, paired = tiles[wname]
            for half in range(2):
                src = pre[64 * half : 64 * half + 64, jo_lo:jo_hi, :].rearrange(
                    "(p i) jo n -> p i jo n", i=2
                )
                pb = 64 * g + 32 * half
                for i in range(2):
                    nc.sync.dma_start(
                        out=paired[pb : pb + 32, i, jo_lo - 3 * g : jo_hi - 3 * g, :],
                        in_=src[:, i, :, :],
                    )

        if jos is not None:
            for jo in jos:
                g = jo // 3
                for wname in names:
                    emit_jo(wname, jo)
                for wname in names:
                    rearrange(wname, g, jo, jo + 1)
            return
        for g in gs:
            for wname in names:
                for jo in range(3 * g, 3 * g + 3):
                    emit_jo(wname, jo)
                rearrange(wname, g, 3 * g, 3 * g + 3)

    def emit_v_proj(b, t8s=None):
        """V projection (fp8 DR); write VE = per-head [V|em] / [em|V] fp8."""
        x8 = x8s[b]
        em_sb = em_sbs[b]
        if VEs[b] is None:
            VEs[b] = ve_pool.tile([P, KO_T, NH, P], FP8, tag="VE", name=f"VE_{b}")
        VE = VEs[b]
        for t8 in (t8s if t8s is not None else range(KO_T)):
            # em columns: even heads cols 64:128, odd heads cols 0:64
            ve_all = VE[:, t8, :, :].rearrange("p (hh two) d -> p hh two d", two=2)
            nc.vector.tensor_scalar_mul(
                out=ve_all[:, :, 0, HD:P],
                in0=ones16,
                scalar1=em_sb[:, t8 : t8 + 1],
            )
            nc.vector.tensor_scalar_mul(
                out=ve_all[:, :, 1, 0:HD],
                in0=ones16,
                scalar1=em_sb[:, t8 : t8 + 1],
            )
            for jh in range(2):
                ps = ps_proj.tile([P, 512], F32, tag="proj")
                for i2 in range(3):
                    lhsT = x8[:, 2 * i2 : 2 * i2 + 2, t8 * P : (t8 + 1) * P]
                    nc.tensor.matmul(
                        ps[:, 0:384],
                        lhsT=lhsT,
                        rhs=w8["v"][:, 2 * i2 : 2 * i2 + 2,
                                    jh * 384 : (jh + 1) * 384],
                        start=(i2 == 0),
                        stop=False,
                        perf_mode=PM.DoubleRow,
                    )
                nc.tensor.matmul(
                    ps[:, 0:384],
                    lhsT=ones_row,
                    rhs=vb16_row[:, jh * 384 : (jh + 1) * 384],
                    start=False,
                    stop=True,
                )
                # heads 6jh..6jh+5 live in psum cols (h-6jh)*64;
                # even heads -> V cols 0:64, odd heads -> V cols 64:128
                ps_v = ps[:, 0:384].rearrange(
                    "p (hh two d) -> p hh two d", two=2, d=HD
                )
                ve_jh = VE[:, t8, 6 * jh : 6 * jh + 6, :].rearrange(
                    "p (hh two) d -> p hh two d", two=2
                )
                nc.vector.tensor_scalar_mul(
                    out=ve_jh[:, :, 0, 0:HD],
                    in0=ps_v[:, :, 0, :],
                    scalar1=em_sb[:, t8 : t8 + 1],
                )
                nc.vector.tensor_scalar_mul(
                    out=ve_jh[:, :, 1, HD:P],
                    in0=ps_v[:, :, 1, :],
                    scalar1=em_sb[:, t8 : t8 + 1],
                )

    def emit_attn(b, hp, qt, cbs=None, direct=False):
        """scores (fp8 DR) -> exp -> PV(+denominator) -> divide, one q-chunk."""
        Q8, K8 = Q8s[b], K8s[b]
        VE = VEs[b]
        if ctxTs[b] is None:
            ctxTs[b] = ctxT_pool.tile([P, 2, 3, S], FP8, tag="ctxT", name=f"ctxT_{b}")
        ctxT = ctxTs[b]
        qsl0 = qt * 512
        pt = pt_pool.tile([P, 2, KO_T, 512], FP8, tag="pt")
        ctxpA = ps_pv.tile([P, 512], F32, tag="pvA")
        ctxpB = ps_pv.tile([P, 512], F32, tag="pvB")
        g, sslot = hp // 3, hp % 3

        def emit_scores(kc):
            for dst, pbase in ((0, 64 * g), (1, 64 * g + 32)):
                sc = ps_sc.tile([P, 2, 512], F32, tag="sc")
                for k2 in range(2):
                    ko = kc * 2 + k2
                    lhsT = K8[pbase : pbase + 32, :, sslot, ko * P : (ko + 1) * P]
                    nc.tensor.matmul(
                        sc[:, k2, :],
                        lhsT=lhsT,
                        rhs=Q8[pbase : pbase + 32, :, sslot, qsl0 : qsl0 + 512],
                        start=True,
                        stop=True,
                        perf_mode=PM.DoubleRow,
                        tile_position=(pbase, 0),
                    )
                nc.scalar.activation(
                    out=pt[:, dst, kc * 2 : kc * 2 + 2, :],
                    in_=sc,
                    func=AF.Exp,
                    scale=EXP_SCALE,
                )

        def emit_pv(kc):
            # DoubleRow: contract 256 keys (ko pair 2kc, 2kc+1) per matmul
            nc.tensor.matmul(
                ctxpA,
                lhsT=VE[:, 2 * kc : 2 * kc + 2, 2 * hp, :],
                rhs=pt[:, 0, 2 * kc : 2 * kc + 2, :],
                start=(kc == 0),
                stop=(kc == KO_T // 2 - 1),
                perf_mode=PM.DoubleRow,
            )
            nc.tensor.matmul(
                ctxpB,
                lhsT=VE[:, 2 * kc : 2 * kc + 2, 2 * hp + 1, :],
                rhs=pt[:, 1, 2 * kc : 2 * kc + 2, :],
                start=(kc == 0),
                stop=(kc == KO_T // 2 - 1),
                perf_mode=PM.DoubleRow,
            )

        # PE order: sc0 sc1 pv0 sc2 pv1 sc3 pv2 pv3 -- keeps the exp stream
        # fed one chunk ahead so ACT never waits on PV matmuls.  cbs inject
        # foreign PE work (output-projection halves) into the slack.
        for kc in range(KO_T // 2):
            emit_scores(kc)
            if cbs and kc in cbs:
                cbs[kc]()
            if kc >= 1:
                emit_pv(kc - 1)
        emit_pv(KO_T // 2 - 1)
        # ctxpA: rows 0:64 = 16*ctx_A, rows 64:128 = 16*den_A
        # ctxpB: rows 0:64 = 16*den_B, rows 64:128 = 16*ctx_B
        rcpT = rcp_pool.tile([P, 512], F32, tag="rcpT")
        nc.vector.reciprocal(out=rcpT[HD:P, :], in_=ctxpA[HD:P, :])
        nc.vector.reciprocal(out=rcpT[0:HD, :], in_=ctxpB[0:HD, :])
        nc.vector.tensor_tensor(
            out=ctxT[0:HD, hp % 2, hp // 2, qsl0 : qsl0 + 512],
            in0=ctxpA[0:HD, :],
            in1=rcpT[HD:P, :],
            op=OP.mult,
        )
        nc.vector.tensor_tensor(
            out=ctxT[HD:P, hp % 2, hp // 2, qsl0 : qsl0 + 512],
            in0=ctxpB[HD:P, :],
            in1=rcpT[0:HD, :],
            op=OP.mult,
        )

    # ---- output projection + residual + layernorm ------------------------
    mv_alls = [None] * BPC
    rstds = [None] * BPC
    s_tiless = [[], []]

    def emit_out_jh(b, t8, jh, state):
        ctxT = ctxTs[b]
        if jh == 0:
            state["xres"] = xres_pool.tile([P, H], F32, tag="xres", name=f"xres_{b}_{t8}")
            nc.sync.dma_start(
                out=state["xres"], in_=hs[b, t8 * P : (t8 + 1) * P, :]
            )
            state["s_t"] = s_pool.tile([P, H], F32, tag="s", name=f"s_{b}_{t8}")
        xres = state["xres"]
        s_t = state["s_t"]
        ps = ps_proj.tile([P, 512], F32, tag="proj")
        for g in range(3):
            nc.tensor.matmul(
                ps[:, 0:384],
                lhsT=ctxT[:, :, g, t8 * P : (t8 + 1) * P],
                rhs=wT_o[:, :, g, jh * 384 : (jh + 1) * 384],
                start=(g == 0),
                stop=False,
                perf_mode=PM.DoubleRow,
            )
        nc.tensor.matmul(
            ps[:, 0:384],
            lhsT=ones_row,
            rhs=ob_row[:, jh * 384 : (jh + 1) * 384],
            start=False,
            stop=True,
        )
        nc.vector.tensor_tensor(
            out=s_t[:, jh * 384 : (jh + 1) * 384],
            in0=ps[:, 0:384],
            in1=xres[:, jh * 384 : (jh + 1) * 384],
            op=OP.add,
        )

    def emit_out(b, t8, pool_gb=False, state=None, rbatch=2):
        if mv_alls[b] is None:
            mv_alls[b] = ln_pool.tile([P, KO_T, 2], F32, tag="mv", name=f"mv_{b}")
            rstds[b] = ln_pool.tile([P, KO_T], F32, tag="rstd", name=f"rstd_{b}")
        mv_all = mv_alls[b]
        rstd = rstds[b]
        s_tiles = s_tiless[b]

        if state is None:
            state = {}
            emit_out_jh(b, t8, 0, state)
            emit_out_jh(b, t8, 1, state)
        s_t = state["s_t"]
        stats = ln_pool.tile([P, 3, 6], F32, tag="stats")
        for sg in range(3):
            nc.vector.bn_stats(
                out=stats[:, sg, :], in_=s_t[:, sg * 256 : (sg + 1) * 256]
            )
        nc.vector.bn_aggr(out=mv_all[:, t8, :], in_=stats)
        s_tiles.append(s_t)

        if t8 % rbatch == rbatch - 1:
            h0 = t8 - (rbatch - 1)
            nc.scalar.activation(
                out=rstd[:, h0 : t8 + 1],
                in_=mv_all[:, h0 : t8 + 1, 1],
                func=AF.Sqrt,
                bias=eps_sb,
                scale=1.0,
            )
            nc.vector.reciprocal(out=rstd[:, h0 : t8 + 1], in_=rstd[:, h0 : t8 + 1])
            for u8 in range(h0, t8 + 1):
                eng = nc.gpsimd if pool_gb else nc.vector
                n_t = n_pool.tile([P, H], F32 if skip_gb else BF16, tag="n")
                eng.tensor_scalar(
                    out=n_t,
                    in0=s_tiles[u8],
                    scalar1=mv_all[:, u8, 0:1],
                    scalar2=rstd[:, u8 : u8 + 1],
                    op0=OP.subtract,
                    op1=OP.mult,
                )
                if skip_gb:
                    # gamma==1, beta==0: the affine step is the identity
                    nc.sync.dma_start(
                        out=out[b, u8 * P : (u8 + 1) * P, :], in_=n_t
                    )
                    continue
                g_t = o_pool.tile([P, H], BF16, tag="g")
                eng.tensor_tensor(out=g_t, in0=n_t, in1=gamma_bc, op=OP.mult)
                o_t = o_pool.tile([P, H], F32, tag="o")
                eng.tensor_tensor(out=o_t, in0=g_t, in1=beta_bc, op=OP.add)
                nc.sync.dma_start(out=out[b, u8 * P : (u8 + 1) * P, :], in_=o_t)

    # ---------------- schedule -------------------------------------------
    emit_mask(0)
    emit_mask(1)
    emit_qk_proj(0, use_act=True)
    stage_rest()
    emit_v_proj(0)

    # b0 attention, qt-major.  b1 projections slot into PE gaps; b0 output
    # tiles start as soon as all heads of a q-chunk are done.
    for hp in range(HP):
        emit_attn(0, hp, 0)
        if hp == 2:
            emit_qk_proj(1, names=("q",), gs=(0,))
        if hp == 3:
            emit_qk_proj(1, names=("q",), gs=(1,))
        if hp == 4:
            emit_qk_proj(1, names=("k",), gs=(0,))
        if hp == 5:
            emit_qk_proj(1, names=("k",), gs=(1,))
    def attn_with_out(ab, hp, qt, ob, t8, pool_gb=False):
        emit_attn(ab, hp, qt)
        emit_out(ob, t8, pool_gb=pool_gb)

    for hp in range(HP):
        if 1 <= hp <= 4:
            attn_with_out(0, hp, 1, 0, hp - 1, pool_gb=True)
        else:
            emit_attn(0, hp, 1)
        if hp == 0:
            emit_v_proj(1)
    for hp in range(HP):
        if hp <= 3:
            attn_with_out(1, hp, 0, 0, 4 + hp, pool_gb=True)
        else:
            emit_attn(1, hp, 0)
    for hp in range(HP):
        if hp <= 3:
            attn_with_out(1, hp, 1, 1, hp, pool_gb=True)
        else:
            emit_attn(1, hp, 1)
    for t8 in range(4, KO_T):
        emit_out(1, t8, rbatch=1, pool_gb=True)


_nc_cache = {}


def _get_nc(skip_gb=True):
    if _nc_cache.get(skip_gb) is None:
        _nc_cache[skip_gb] = build_bass(skip_gb)
    return _nc_cache[skip_gb]


def kernel(**inputs):
    import ml_dtypes
    from concourse.bass_utils import run_bass_kernel_spmd

    E4M3 = ml_dtypes.float8_e4m3
    BF = ml_dtypes.bfloat16

    def wt8(w):
        # w [H, H] (torch Linear weight): lhsT layout [128, IO_T, H] of 16*w^T
        wt = np.asarray(w, np.float32).T.reshape(IO_T, P, H).transpose(1, 0, 2)
        return np.ascontiguousarray((wt * WSCALE).astype(E4M3))

    def _wto_dr(ow):
        p = np.arange(P)[:, None, None]
        i = np.arange(2)[None, :, None]
        g = np.arange(3)[None, None, :]
        f = 64 * (4 * g + 2 * i + (p >= 64)) + (p % 64)  # [128, 2, 3]
        return np.ascontiguousarray((WSCALE * ow.T.astype(np.float32))[f, :].astype(E4M3))

    hs = np.asarray(inputs["hidden_states"], np.float32)
    mask = np.asarray(inputs["attention_mask"], np.float32).reshape(B, S)
    # x^T fp8: [B, 128, IO_T, S]
    x8 = np.ascontiguousarray(
        hs.transpose(0, 2, 1).reshape(B, IO_T, P, S).transpose(0, 2, 1, 3).astype(E4M3)
    )
    shared = {
        "w8q": wt8(inputs["qw"]),
        "w8k": wt8(inputs["kw"]),
        "w8v": wt8(inputs["vw"]),
        # o-proj lhsT pairing for DoubleRow: contraction row (p, i) in group g
        # is ctx feature f = 64*(4g+2i+(p>=64)) + p%64 (matches ctxT layout
        # [p, i=hp%2, g=hp//2] with head A on partitions 0:64, B on 64:128).
        "wTo": _wto_dr(np.asarray(inputs["ow"], np.float32)),
        "qb16d": np.ascontiguousarray(
            (np.asarray(inputs["qb"], np.float32) * WSCALE).reshape(IO_T, P).T
        ),
        "kb16d": np.ascontiguousarray(
            (np.asarray(inputs["kb"], np.float32) * WSCALE).reshape(IO_T, P).T
        ),
        "vb16d": np.ascontiguousarray(
            (np.asarray(inputs["vb"], np.float32) * WSCALE).astype(BF)
        ),
        "obd": np.ascontiguousarray(
            (np.asarray(inputs["ob"], np.float32) * WSCALE).astype(BF)
        ),
        "gamma": np.ascontiguousarray(np.asarray(inputs["gamma"], np.float32)),
        "beta": np.ascontiguousarray(np.asarray(inputs["beta"], np.float32)),
    }
    # residual staged pre-scaled by 16 so the fp8 o-proj's 16x weight scale
    # cancels inside the (scale-invariant) layernorm: s = 16*(dense+ob+hs)
    hs16 = hs * WSCALE
    in_maps = []
    for c in range(NCORES):
        m = dict(shared)
        m["hs"] = np.ascontiguousarray(hs16[c * BPC : (c + 1) * BPC])
        m["x8d"] = np.ascontiguousarray(x8[c * BPC : (c + 1) * BPC])
        m["msk"] = np.ascontiguousarray(mask[c * BPC : (c + 1) * BPC])
        in_maps.append(m)

    # A rare per-process DMA race can corrupt a core's staging buffer, which
    # surfaces as NaN/Inf.  Sticky per module load: rebuild after 2 failures.
    skip_gb = bool(
        np.all(shared["gamma"] == 1.0) and np.all(shared["beta"] == 0.0)
    )
    out = None
    for attempt in range(6):
        res = run_bass_kernel_spmd(
            _get_nc(skip_gb), in_maps, core_ids=list(range(NCORES))
        )
        out = np.concatenate([res.results[c]["out"] for c in range(NCORES)], axis=0)
        if np.isfinite(out).all():
            break
        if attempt >= 1:
            _nc_cache[skip_gb] = None
    return out



# revision 56
# speedup vs baseline: 1.0672x; 1.0146x over previous
"""Trainium2 Bass kernel for BertAttention (B=16, S=1024, H=768, 12 heads).

Data-parallel over batch across 8 NeuronCores (2 batch rows per core).

Host side (in kernel()): weights are pre-transposed to lhsT layout,
pre-scaled by 16 and quantized to fp8e4; x is pre-transposed/quantized
likewise.  hs stays f32 for the residual + layernorm path.

Per-core device kernel:
  - Q/K/V projections as fp8e4 DoubleRow matmuls (0.5 cyc/row, 256-deep
    contraction per matmul).
  - scores as fp8e4 DoubleRow matmuls with Q/K in a feature-paired
    [32, 2, tok] SBUF layout produced by an SBUF->SBUF DMA rearrange
    (4 half-heads per 128 partitions, 3 head-pairs along the free axis).
  - exp on ACT engine with the 1/(8*256) scale folded in, writing fp8
    probs directly; the additive mask is applied exactly as a
    multiplicative exp(mask) folded into the V rows and denominator.
  - softmax denominator folded INTO the PV matmul: lhsT = [V_h | em] for
    even heads and [em | V_h] for odd heads, so denominators accumulate on
    the other 64 PSUM partitions for free.  Division via DVE reciprocal +
    multiply with mixed-partition-offset operands (PSUM in0 + SBUF in1).
  - output projection in bf16 + residual add + LayerNorm
    (bn_stats/bn_aggr, Sqrt batched per t8 pair, gamma in bf16).
  - schedule keeps the ACT engine (the bottleneck at ~216us busy)
    saturated: PE emits scores one k-chunk ahead of PV, batch row b1's
    projections ride in b0's attention slack, and output tiles interleave
    into the following attention loop.

Workaround: this container's walrus accepts only ONE sync wait per
instruction; a post-pass splits multi-wait instructions into single-wait
NOPs.
"""

import numpy as np

import concourse.bass as bass
import concourse.mybir as mybir
import concourse.tile as tile

P = 128
H = 768
NH = 12
HD = 64
S = 1024
B = 16
NCORES = 8
BPC = B // NCORES  # 2
IO_T = H // P      # 6
KO_T = S // P      # 8
HP = NH // 2       # 6 head pairs (one per 128-feature block)
WSCALE = 16.0
EXP_SCALE = 1.0 / (8.0 * WSCALE * WSCALE)  # 1/sqrt(64) / (16*16)
LN_EPS = 1e-12

F32 = mybir.dt.float32
BF16 = mybir.dt.bfloat16
FP8 = mybir.dt.float8e4
AF = mybir.ActivationFunctionType
OP = mybir.AluOpType
PM = mybir.MatmulPerfMode


def _split_multi_waits(nc):
    """walrus here rejects >1 sync wait per instruction; hoist extras into
    single-wait NOPs on the same engine immediately before."""
    n = 0
    for blk in nc.m.functions[0].blocks:
        insts = blk.instructions
        new = []
        changed = False
        for inst in insts:
            si = inst.sync_info
            waits = list(si.on_wait) if si and si.on_wait else []
            if len(waits) > 1:
                changed = True
                for k, w in enumerate(waits[:-1]):
                    n += 1
                    new.append(
                        mybir.InstNoOp(
                            name=f"ws-{blk.name}-{inst.name}-{k}",
                            engine=inst.engine,
                            sync_info=mybir.SyncInfo(on_wait=[w], on_update=[]),
                        )
                    )
                inst.sync_info = mybir.SyncInfo(
                    on_wait=[waits[-1]], on_update=list(si.on_update)
                )
            new.append(inst)
        if changed:
            blk.instructions = new
    return n


def _bcast_ap(ap, parts=P):
    return bass.AP(tensor=ap.tensor, offset=ap.offset, ap=[[0, parts]] + list(ap.ap))


def build_bass(skip_gb=False):
    nc = bass.Bass()

    # weights / x arrive pre-transposed, pre-scaled (x16) and pre-quantized
    # to fp8e4 from the host; hs stays f32 for the residual + layernorm path.
    hs = nc.declare_dram_parameter("hs", [BPC, S, H], F32, isOutput=False)
    x8d = nc.declare_dram_parameter("x8d", [BPC, P, IO_T, S], FP8, isOutput=False)
    scon = nc.declare_dram_parameter("scon", [P, 28], F32, isOutput=False)
    w8qk = nc.declare_dram_parameter("w8qk", [P, 2, IO_T, H], FP8, isOutput=False)
    w8v = nc.declare_dram_parameter("w8v", [P, IO_T, H], FP8, isOutput=False)
    wTo = nc.declare_dram_parameter("wTo", [P, 2, 3, H], FP8, isOutput=False)
    vb16d = nc.declare_dram_parameter("vb16d", [H], BF16, isOutput=False)
    obd = nc.declare_dram_parameter("obd", [H], BF16, isOutput=False)
    gamma = nc.declare_dram_parameter("gamma", [H], F32, isOutput=False)
    beta = nc.declare_dram_parameter("beta", [H], F32, isOutput=False)
    out = nc.declare_dram_parameter("out", [BPC, S, H], F32, isOutput=True)

    from contextlib import ExitStack

    with tile.TileContext(nc) as tc:
        with ExitStack() as ctx:
            _build_tile(
                ctx, tc, nc, hs, x8d, scon, w8qk, w8v, wTo,
                vb16d, obd, gamma, beta, out, skip_gb
            )

    _split_multi_waits(nc)
    return nc


def _build_tile(ctx, tc, nc, hs, x8d, scon, w8qk, w8v, wTo,
                vb16d, obd, gamma, beta, out, skip_gb=False):
    dram = ctx.enter_context(tc.tile_pool(name="dram", bufs=1, space="DRAM"))
    consts = ctx.enter_context(tc.tile_pool(name="consts", bufs=1))
    perb = ctx.enter_context(tc.tile_pool(name="perb", bufs=2))
    x8_pool = ctx.enter_context(tc.tile_pool(name="x8", bufs=2))
    pre_pool = ctx.enter_context(tc.tile_pool(name="pre", bufs=2))
    qk8_pool = ctx.enter_context(tc.tile_pool(name="qk8", bufs=4))
    ve_pool = ctx.enter_context(tc.tile_pool(name="ve", bufs=2))
    pt_pool = ctx.enter_context(tc.tile_pool(name="pt", bufs=5))
    rcp_pool = ctx.enter_context(tc.tile_pool(name="rcp", bufs=2))
    ctxT_pool = ctx.enter_context(tc.tile_pool(name="ctxT", bufs=2))
    xres_pool = ctx.enter_context(tc.tile_pool(name="xres", bufs=2))
    s_pool = ctx.enter_context(tc.tile_pool(name="s", bufs=4))
    n_pool = ctx.enter_context(tc.tile_pool(name="n", bufs=2))
    o_pool = ctx.enter_context(tc.tile_pool(name="o", bufs=2))
    ln_pool = ctx.enter_context(tc.tile_pool(name="ln", bufs=4))

    ps_proj = ctx.enter_context(tc.tile_pool(name="psp", bufs=2, space="PSUM"))
    ps_sc = ctx.enter_context(tc.tile_pool(name="pssc", bufs=2, space="PSUM"))
    ps_pv = ctx.enter_context(tc.tile_pool(name="pspv", bufs=1, space="PSUM"))

    # ---------------- constants / weight staging --------------------------
    w8 = {}
    x8s = []
    for b in range(BPC):
        x8s.append(x8_pool.tile([P, IO_T, S], FP8, tag="x8", name=f"x8_{b}"))
    w8qk_sb = consts.tile([P, 2, IO_T, H], FP8, tag="w8qk_sb", name="w8qk_sb")
    w8["q"] = w8qk_sb[:, 0]
    w8["k"] = w8qk_sb[:, 1]
    w8["v"] = consts.tile([P, IO_T, H], FP8, tag="w8_v", name="w8_v")
    wT_o = consts.tile([P, 2, 3, H], FP8, tag="wT_o", name="wT_o")
    # DMA transfers serialize on the (modeled) DMA engine block, so the
    # startup order IS the critical path: masks first (unblocks the em exp),
    # then x8 token-halves, then only the jo0 column slices of wq/wk so the
    # first head-pair's projections start ~4us earlier than a full-w load.
    scon_sb = consts.tile([P, 28], F32, tag="scon_sb", name="scon_sb")
    nc.sync.dma_start(out=w8qk_sb[:, :, :, 0:P], in_=w8qk[:, :, :, 0:P])
    nc.sync.dma_start(out=scon_sb, in_=scon[:, :])
    nc.sync.dma_start(out=x8s[0][:, :, 0:512], in_=x8d[0][:, :, 0:512])
    nc.sync.dma_start(out=x8s[0][:, :, 512:S], in_=x8d[0][:, :, 512:S])
    nc.scalar.dma_start(out=w8qk_sb[:, :, :, P:H], in_=w8qk[:, :, :, P:H])
    nc.gpsimd.dma_start(out=w8["v"], in_=w8v[:, :, :])

    def stage_rest():
        nc.sync.dma_start(out=x8s[1], in_=x8d[:, :, :, :][1])
        nc.sync.dma_start(out=wT_o, in_=wTo[:, :, :, :])
        nc.sync.dma_start(out=ob_row, in_=obd[:][None, :])
        nc.gpsimd.dma_start(out=gamma_bc, in_=_bcast_ap(gamma[:]))
        nc.gpsimd.dma_start(out=beta_bc, in_=_bcast_ap(beta[:]))

    qb16 = scon_sb[:, 0:IO_T]
    kb16 = scon_sb[:, IO_T : 2 * IO_T]
    vb16_row = consts.tile([1, H], BF16, tag="vb16_row")
    nc.scalar.dma_start(out=vb16_row, in_=vb16d[:][None, :])
    ob_row = consts.tile([1, H], BF16, tag="ob_row")
    gamma_bc = consts.tile([P, H], BF16, tag="gamma_bc")
    beta_bc = consts.tile([P, H], F32, tag="beta_bc")

    eps_sb = consts.tile([P, 1], F32, tag="eps")
    nc.vector.memset(eps_sb, LN_EPS)
    ones_row = consts.tile([1, P], BF16, tag="ones_row")
    nc.vector.memset(ones_row, 1.0)
    ones16 = consts.tile([P, 6, 2, HD], BF16, tag="ones16")
    nc.vector.memset(ones16, WSCALE)

    # ---------------- per-b state ----------------------------------------
    qk_tiles = {}
    em_sbs = [None] * BPC
    Q8s = [None] * BPC
    K8s = [None] * BPC
    VEs = [None] * BPC
    ctxTs = [None] * BPC

    def emit_mask(b):
        mask_sb = scon_sb[:, 12 + KO_T * b : 12 + KO_T * (b + 1)]
        em_sbs[b] = perb.tile([P, KO_T], F32, tag="em", name=f"em_{b}")
        nc.scalar.activation(out=em_sbs[b], in_=mask_sb, func=AF.Exp)

    def emit_qk_proj(b, names=("q", "k"), use_act=False, gs=(0, 1), jos=None,
                     tts=(0, 1), do_mm=True, do_rearr=True):
        """Q/K projections (fp8 DR) -> fp8 pre tiles -> DMA pair-rearrange.

        Emitted g-outer (3-jo group), tensor-inner, so the first head-pairs
        of BOTH Q and K are ready before later groups.  use_act routes the
        PSUM->fp8+bias copies to the ACT engine; default is DVE.  jos
        (absolute jo indices) selects jo-granular emission + rearrange so a
        single head-pair's Q/K can be staged on the startup critical path."""
        x8 = x8s[b]
        cfg = {"q": (qb16, Q8s), "k": (kb16, K8s)}
        tiles = {}
        for wname in names:
            if (b, wname) not in qk_tiles:
                qk_tiles[(b, wname)] = (
                    pre_pool.tile([P, IO_T, S], FP8, tag="pre", name=f"{wname}pre_{b}"),
                    qk8_pool.tile([P, 2, 3, S], FP8, tag="qk8", name=f"{wname}8_{b}"),
                )
            tiles[wname] = qk_tiles[(b, wname)]
            cfg[wname][1][b] = tiles[wname][1]

        def emit_jo(wname, jo):
            bias, _ = cfg[wname]
            pre, paired = tiles[wname]
            for tt in tts:
                ps = ps_proj.tile([P, 512], F32, tag="proj")
                for i2 in range(3):
                    lhsT = w8[wname][:, 2 * i2 : 2 * i2 + 2, jo * P : (jo + 1) * P]
                    nc.tensor.matmul(
                        ps,
                        lhsT=lhsT,
                        rhs=x8[:, 2 * i2 : 2 * i2 + 2, tt * 512 : (tt + 1) * 512],
                        start=(i2 == 0),
                        stop=(i2 == 2),
                        perf_mode=PM.DoubleRow,
                    )
                if use_act is True or (use_act == "tt0" and tt == 0):
                    nc.scalar.activation(
                        out=pre[:, jo, tt * 512 : (tt + 1) * 512],
                        in_=ps,
                        func=AF.Identity,
                        bias=bias[:, jo : jo + 1],
                    )
                else:
                    nc.vector.tensor_scalar_add(
                        out=pre[:, jo, tt * 512 : (tt + 1) * 512],
                        in0=ps,
                        scalar1=bias[:, jo : jo + 1],
                    )

        def rearrange(wname, g, jo_lo, jo_hi):
            # pair-rearrange: head-pair hp -> (g = hp//3, s = hp%3);
            # half-head (hp, A) at partitions 64g..64g+32, (hp, B) at +32.
            # paired[p, i, s, n]: feature 2(p%32)+i of that half-head.
            pre, paired = tiles[wname]
            eng = nc.sync
            for half in range(2):
                src = pre[64 * half : 64 * half + 64, jo_lo:jo_hi, :].rearrange(
                    "(p i) jo n -> p i jo n", i=2
                )
                pb = 64 * g + 32 * half
                for i in range(2):
                    eng.dma_start(
                        out=paired[pb : pb + 32, i, jo_lo - 3 * g : jo_hi - 3 * g, :],
                        in_=src[:, i, :, :],
                    )

        if jos is not None:
            if do_mm:
                for jo in jos:
                    for wname in names:
                        emit_jo(wname, jo)
            if not do_rearr:
                return
            # rearrange contiguous runs (within one g) per tensor
            run = [jos[0], jos[0] + 1]
            runs = [run]
            for jo in jos[1:]:
                if jo == run[1] and jo // 3 == run[0] // 3:
                    run[1] = jo + 1
                else:
                    run = [jo, jo + 1]
                    runs.append(run)
            for wname in names:
                for lo, hi in runs:
                    rearrange(wname, lo // 3, lo, hi)
            return
        for g in gs:
            for wname in names:
                for jo in range(3 * g, 3 * g + 3):
                    emit_jo(wname, jo)
                rearrange(wname, g, 3 * g, 3 * g + 3)

    def emit_v_proj(b, t8s=None, use_act=False):
        """V projection (fp8 DR); write VE = per-head [V|em] / [em|V] fp8.

        use_act routes the PSUM->fp8 em-scaled copies to the ACT engine
        (only sensible pre-stream, when ACT is otherwise idle)."""
        x8 = x8s[b]
        em_sb = em_sbs[b]
        if VEs[b] is None:
            VEs[b] = ve_pool.tile([P, KO_T, NH, P], FP8, tag="VE", name=f"VE_{b}")
        VE = VEs[b]
        for t8 in (t8s if t8s is not None else range(KO_T)):
            # every head's VE row is [V | em]: em columns 64:128
            ve_all = VE[:, t8, :, :].rearrange("p (hh two) d -> p hh two d", two=2)
            nc.vector.tensor_scalar_mul(
                out=ve_all[:, :, :, HD:P],
                in0=ones16,
                scalar1=em_sb[:, t8 : t8 + 1],
            )
            for jh in range(2):
                ps = ps_proj.tile([P, 512], F32, tag="proj")
                for i2 in range(3):
                    lhsT = x8[:, 2 * i2 : 2 * i2 + 2, t8 * P : (t8 + 1) * P]
                    nc.tensor.matmul(
                        ps[:, 0:384],
                        lhsT=lhsT,
                        rhs=w8["v"][:, 2 * i2 : 2 * i2 + 2,
                                    jh * 384 : (jh + 1) * 384],
                        start=(i2 == 0),
                        stop=False,
                        perf_mode=PM.DoubleRow,
                    )
                nc.tensor.matmul(
                    ps[:, 0:384],
                    lhsT=ones_row,
                    rhs=vb16_row[:, jh * 384 : (jh + 1) * 384],
                    start=False,
                    stop=True,
                )
                # heads 6jh..6jh+5 live in psum cols (h-6jh)*64; every
                # head's V block goes to VE cols 0:64 (em lives in 64:128)
                ps_v = ps[:, 0:384].rearrange(
                    "p (hh two d) -> p hh two d", two=2, d=HD
                )
                ve_jh = VE[:, t8, 6 * jh : 6 * jh + 6, :].rearrange(
                    "p (hh two) d -> p hh two d", two=2
                )
                if use_act:
                    nc.scalar.activation(
                        out=ve_jh[:, :, :, 0:HD], in_=ps_v,
                        func=AF.Copy, scale=em_sb[:, t8 : t8 + 1],
                    )
                else:
                    nc.vector.tensor_scalar_mul(
                        out=ve_jh[:, :, :, 0:HD],
                        in0=ps_v,
                        scalar1=em_sb[:, t8 : t8 + 1],
                    )

    attn_pts = {}

    def emit_attn(b, hp, qt, cbs=None, defer_pv=False, direct=False, split=False,
                  kcs=None):
        """scores (fp8 DR) -> exp -> PV(+denominator) -> divide, one q-chunk.

        defer_pv emits all scores before any PV matmul: used for the first
        unit(s), where PV would stall on VE writes at the PE queue head and
        block later score matmuls (head-of-line) from feeding the ACT exp
        stream.  direct reads scores straight from the un-paired pre tiles
        (non-DR fp8, 2x PE cost) to skip the pair-rearrange on the startup
        critical path.  split=True emits only scores+exp and returns a
        finisher that emits PV+divide (call it later, once VE is staged)."""
        Q8, K8 = Q8s[b], K8s[b]
        if ctxTs[b] is None:
            ctxTs[b] = ctxT_pool.tile([P, 2, 3, S], FP8, tag="ctxT", name=f"ctxT_{b}")
        ctxT = ctxTs[b]
        qsl0 = qt * 512
        if (b, hp, qt) in attn_pts:
            pt = attn_pts[(b, hp, qt)]
        else:
            pt = pt_pool.tile([P, 2, KO_T, 512], FP8, tag="pt",
                              name=f"pt_{b}_{hp}_{qt}")
            attn_pts[(b, hp, qt)] = pt
        g, sslot = hp // 3, hp % 3

        def emit_scores(kc):
            for dst, pbase in ((0, 64 * g), (1, 64 * g + 32)):
                sc = ps_sc.tile([P, 2, 512], F32, tag="sc")
                for k2 in range(2):
                    ko = kc * 2 + k2
                    if direct:
                        preq = qk_tiles[(b, "q")][0]
                        prek = qk_tiles[(b, "k")][0]
                        db = 64 * dst
                        nc.tensor.matmul(
                            sc[:, k2, :],
                            lhsT=prek[db : db + 64, hp, ko * P : (ko + 1) * P],
                            rhs=preq[db : db + 64, hp, qsl0 : qsl0 + 512],
                            start=True,
                            stop=True,
                            tile_position=(db, 0),
                        )
                        continue
                    lhsT = K8[pbase : pbase + 32, :, sslot, ko * P : (ko + 1) * P]
                    nc.tensor.matmul(
                        sc[:, k2, :],
                        lhsT=lhsT,
                        rhs=Q8[pbase : pbase + 32, :, sslot, qsl0 : qsl0 + 512],
                        start=True,
                        stop=True,
                        perf_mode=PM.DoubleRow,
                        tile_position=(pbase, 0),
                    )
                nc.scalar.activation(
                    out=pt[:, dst, kc * 2 : kc * 2 + 2, :],
                    in_=sc,
                    func=AF.Exp,
                    scale=EXP_SCALE,
                )

        ctxp = [None, None]

        def alloc_pv():
            ctxp[0] = ps_pv.tile([P, 512], F32, tag="pvA", name=f"pvA_{b}_{hp}_{qt}")
            ctxp[1] = ps_pv.tile([P, 512], F32, tag="pvB", name=f"pvB_{b}_{hp}_{qt}")

        def emit_pv(kc):
            # DoubleRow: contract 256 keys (ko pair 2kc, 2kc+1) per matmul
            VE = VEs[b]
            nc.tensor.matmul(
                ctxp[0],
                lhsT=VE[:, 2 * kc : 2 * kc + 2, 2 * hp, :],
                rhs=pt[:, 0, 2 * kc : 2 * kc + 2, :],
                start=(kc == 0),
                stop=(kc == KO_T // 2 - 1),
                perf_mode=PM.DoubleRow,
            )
            nc.tensor.matmul(
                ctxp[1],
                lhsT=VE[:, 2 * kc : 2 * kc + 2, 2 * hp + 1, :],
                rhs=pt[:, 1, 2 * kc : 2 * kc + 2, :],
                start=(kc == 0),
                stop=(kc == KO_T // 2 - 1),
                perf_mode=PM.DoubleRow,
            )

        def emit_div():
            # ctxp[d]: rows 0:64 = 16*ctx, rows 64:128 = 16*den (both dsts).
            # Partition-offset operand only ever on in1 (the proven pattern).
            rcpT = rcp_pool.tile([P, 2, 512], F32, tag="rcpT")
            nc.vector.reciprocal(out=rcpT[HD:P, 0, :], in_=ctxp[0][HD:P, :])
            nc.vector.reciprocal(out=rcpT[HD:P, 1, :], in_=ctxp[1][HD:P, :])
            nc.vector.tensor_tensor(
                out=ctxT[0:HD, hp % 2, hp // 2, qsl0 : qsl0 + 512],
                in0=ctxp[0][0:HD, :],
                in1=rcpT[HD:P, 0, :],
                op=OP.mult,
            )
            nc.vector.tensor_tensor(
                out=ctxT[HD:P, hp % 2, hp // 2, qsl0 : qsl0 + 512],
                in0=rcpT[HD:P, 1, :],
                in1=ctxp[1][0:HD, :],
                op=OP.mult,
            )

        if split:
            for kc in (kcs if kcs is not None else range(KO_T // 2)):
                emit_scores(kc)

            def finish():
                alloc_pv()
                for kc in range(KO_T // 2):
                    emit_pv(kc)
                emit_div()

            return finish

        alloc_pv()
        # PE order: sc0 sc1 pv0 sc2 pv1 sc3 pv2 pv3 -- keeps the exp stream
        # fed one chunk ahead so ACT never waits on PV matmuls.  cbs inject
        # foreign PE work (output-projection halves) into the slack.
        for kc in range(KO_T // 2):
            emit_scores(kc)
            if cbs and kc in cbs:
                cbs[kc]()
            if not defer_pv and kc >= 1:
                emit_pv(kc - 1)
        if defer_pv:
            for kc in range(KO_T // 2):
                emit_pv(kc)
        else:
            emit_pv(KO_T // 2 - 1)
        emit_div()

    # ---- output projection + residual + layernorm ------------------------
    mv_alls = [None] * BPC
    rstds = [None] * BPC
    s_tiless = [[], []]

    def emit_out_jh(b, t8, jh, state):
        ctxT = ctxTs[b]
        if jh == 0:
            state["xres"] = xres_pool.tile([P, H], F32, tag="xres", name=f"xres_{b}_{t8}")
            nc.sync.dma_start(
                out=state["xres"], in_=hs[b, t8 * P : (t8 + 1) * P, :]
            )
            state["s_t"] = s_pool.tile([P, H], F32, tag="s", name=f"s_{b}_{t8}")
        xres = state["xres"]
        s_t = state["s_t"]
        ps = ps_proj.tile([P, 512], F32, tag="proj")
        for g in range(3):
            nc.tensor.matmul(
                ps[:, 0:384],
                lhsT=ctxT[:, :, g, t8 * P : (t8 + 1) * P],
                rhs=wT_o[:, :, g, jh * 384 : (jh + 1) * 384],
                start=(g == 0),
                stop=False,
                perf_mode=PM.DoubleRow,
            )
        nc.tensor.matmul(
            ps[:, 0:384],
            lhsT=ones_row,
            rhs=ob_row[:, jh * 384 : (jh + 1) * 384],
            start=False,
            stop=True,
        )
        nc.vector.tensor_tensor(
            out=s_t[:, jh * 384 : (jh + 1) * 384],
            in0=ps[:, 0:384],
            in1=xres[:, jh * 384 : (jh + 1) * 384],
            op=OP.add,
        )

    def emit_out(b, t8, pool_gb=False, state=None, rbatch=2):
        if mv_alls[b] is None:
            mv_alls[b] = ln_pool.tile([P, KO_T, 2], F32, tag="mv", name=f"mv_{b}")
            rstds[b] = ln_pool.tile([P, KO_T], F32, tag="rstd", name=f"rstd_{b}")
        mv_all = mv_alls[b]
        rstd = rstds[b]
        s_tiles = s_tiless[b]

        if state is None:
            state = {}
            emit_out_jh(b, t8, 0, state)
            emit_out_jh(b, t8, 1, state)
        s_t = state["s_t"]
        stats = ln_pool.tile([P, 3, 6], F32, tag="stats")
        for sg in range(3):
            nc.vector.bn_stats(
                out=stats[:, sg, :], in_=s_t[:, sg * 256 : (sg + 1) * 256]
            )
        nc.vector.bn_aggr(out=mv_all[:, t8, :], in_=stats)
        s_tiles.append(s_t)

        if t8 % rbatch == rbatch - 1:
            h0 = t8 - (rbatch - 1)
            nc.scalar.activation(
                out=rstd[:, h0 : t8 + 1],
                in_=mv_all[:, h0 : t8 + 1, 1],
                func=AF.Sqrt,
                bias=eps_sb,
                scale=1.0,
            )
            nc.vector.reciprocal(out=rstd[:, h0 : t8 + 1], in_=rstd[:, h0 : t8 + 1])
            for u8 in range(h0, t8 + 1):
                eng = nc.gpsimd if pool_gb else nc.vector
                n_t = n_pool.tile([P, H], F32 if skip_gb else BF16, tag="n")
                eng.tensor_scalar(
                    out=n_t,
                    in0=s_tiles[u8],
                    scalar1=mv_all[:, u8, 0:1],
                    scalar2=rstd[:, u8 : u8 + 1],
                    op0=OP.subtract,
                    op1=OP.mult,
                )
                if skip_gb:
                    # gamma==1, beta==0: the affine step is the identity
                    nc.sync.dma_start(
                        out=out[b, u8 * P : (u8 + 1) * P, :], in_=n_t
                    )
                    continue
                g_t = o_pool.tile([P, H], BF16, tag="g")
                eng.tensor_tensor(out=g_t, in0=n_t, in1=gamma_bc, op=OP.mult)
                o_t = o_pool.tile([P, H], F32, tag="o")
                eng.tensor_tensor(out=o_t, in0=g_t, in1=beta_bc, op=OP.add)
                nc.sync.dma_start(out=out[b, u8 * P : (u8 + 1) * P, :], in_=o_t)

    # ---------------- schedule -------------------------------------------
    # Startup: stage q/k jo-granular so hp0's attention starts after jo0;
    # v-proj interleaves so the DVE does jo-adds and VE writes just-in-time.
    # hp0-qt0 runs "direct" off the un-paired jo0 pre tiles the moment the
    # jo0 tt0 projections land (kc0/kc1 only need keys 0:512 and the qt0
    # queries 0:512 = tt0); tt1 + kc2/kc3 follow.  PV+divide deferred until
    # VE is staged.
    emit_qk_proj(0, jos=(0,), use_act=True, tts=(0,), do_rearr=False)
    emit_attn(0, 0, 0, direct=True, split=True, kcs=(0, 1))
    emit_qk_proj(0, jos=(0,), tts=(1,), do_rearr=False)
    finish_hp0 = emit_attn(0, 0, 0, direct=True, split=True, kcs=(2, 3))
    emit_mask(0)
    emit_mask(1)
    emit_qk_proj(0, jos=(1, 2))
    emit_qk_proj(0, jos=(3,))
    emit_qk_proj(0, jos=(4, 5))
    emit_v_proj(0, t8s=(0, 1))
    emit_v_proj(0, t8s=(2, 3))
    emit_v_proj(0, t8s=(4, 5))
    emit_v_proj(0, t8s=(6, 7))
    emit_qk_proj(0, jos=(0,), do_mm=False)
    stage_rest()

    # b0 attention qt0, unit-pipelined: unit N's scores are emitted before
    # unit N-1's PV+divide so a VE-staging stall at the PE queue head never
    # starves the exp stream.  b1 projections slot into the PE slack.
    prev_fin = finish_hp0
    for hp in range(1, HP):
        fin = emit_attn(0, hp, 0, split=True)
        prev_fin()
        prev_fin = fin
        if hp == 3:
            emit_qk_proj(1, names=("q",), gs=(0,))
        if hp == 4:
            emit_qk_proj(1, names=("q",), gs=(1,))
        if hp == 5:
            emit_qk_proj(1, names=("k",), gs=(0,))
    prev_fin()
    def attn_with_out(ab, hp, qt, ob, t8, pool_gb=False):
        emit_attn(ab, hp, qt)
        emit_out(ob, t8, pool_gb=pool_gb)

    for hp in range(HP):
        if 1 <= hp <= 4:
            attn_with_out(0, hp, 1, 0, hp - 1, pool_gb=True)
        else:
            emit_attn(0, hp, 1)
        if hp == 0:
            emit_qk_proj(1, names=("k",), gs=(1,))
            emit_v_proj(1)
    for hp in range(HP):
        if hp <= 3:
            attn_with_out(1, hp, 0, 0, 4 + hp, pool_gb=True)
        else:
            emit_attn(1, hp, 0)
    for hp in range(HP):
        if hp <= 3:
            attn_with_out(1, hp, 1, 1, hp, pool_gb=True)
        else:
            emit_attn(1, hp, 1)
    for t8 in range(4, KO_T):
        emit_out(1, t8, rbatch=1, pool_gb=True)


_nc_cache = {}


def _get_nc(skip_gb=True):
    if _nc_cache.get(skip_gb) is None:
        _nc_cache[skip_gb] = build_bass(skip_gb)
    return _nc_cache[skip_gb]


def kernel(**inputs):
    import ml_dtypes
    from concourse.bass_utils import run_bass_kernel_spmd

    E4M3 = ml_dtypes.float8_e4m3
    BF = ml_dtypes.bfloat16

    def wt8(w):
        # w [H, H] (torch Linear weight): lhsT layout [128, IO_T, H] of 16*w^T
        wt = np.asarray(w, np.float32).T.reshape(IO_T, P, H).transpose(1, 0, 2)
        return np.ascontiguousarray((wt * WSCALE).astype(E4M3))

    def _wto_dr(ow):
        p = np.arange(P)[:, None, None]
        i = np.arange(2)[None, :, None]
        g = np.arange(3)[None, None, :]
        f = 64 * (4 * g + 2 * i + (p >= 64)) + (p % 64)  # [128, 2, 3]
        return np.ascontiguousarray((WSCALE * ow.T.astype(np.float32))[f, :].astype(E4M3))

    hs = np.asarray(inputs["hidden_states"], np.float32)
    mask = np.asarray(inputs["attention_mask"], np.float32).reshape(B, S)
    # x^T fp8: [B, 128, IO_T, S]
    x8 = np.ascontiguousarray(
        hs.transpose(0, 2, 1).reshape(B, IO_T, P, S).transpose(0, 2, 1, 3).astype(E4M3)
    )
    shared = {
        "w8qk": np.ascontiguousarray(
            np.stack([wt8(inputs["qw"]), wt8(inputs["kw"])], axis=1)
        ),
        "w8v": wt8(inputs["vw"]),
        # o-proj lhsT pairing for DoubleRow: contraction row (p, i) in group g
        # is ctx feature f = 64*(4g+2i+(p>=64)) + p%64 (matches ctxT layout
        # [p, i=hp%2, g=hp//2] with head A on partitions 0:64, B on 64:128).
        "wTo": _wto_dr(np.asarray(inputs["ow"], np.float32)),

        "vb16d": np.ascontiguousarray(
            (np.asarray(inputs["vb"], np.float32) * WSCALE).astype(BF)
        ),
        "obd": np.ascontiguousarray(
            (np.asarray(inputs["ob"], np.float32) * WSCALE).astype(BF)
        ),
        "gamma": np.ascontiguousarray(np.asarray(inputs["gamma"], np.float32)),
        "beta": np.ascontiguousarray(np.asarray(inputs["beta"], np.float32)),
    }
    # residual staged pre-scaled by 16 so the fp8 o-proj's 16x weight scale
    # cancels inside the (scale-invariant) layernorm: s = 16*(dense+ob+hs)
    hs16 = hs * WSCALE
    qb16 = (np.asarray(inputs["qb"], np.float32) * WSCALE).reshape(IO_T, P).T
    kb16 = (np.asarray(inputs["kb"], np.float32) * WSCALE).reshape(IO_T, P).T
    in_maps = []
    for c in range(NCORES):
        m = dict(shared)
        m["hs"] = np.ascontiguousarray(hs16[c * BPC : (c + 1) * BPC])
        msk_c = mask[c * BPC : (c + 1) * BPC].reshape(BPC, KO_T, P)
        m["scon"] = np.ascontiguousarray(np.concatenate(
            [qb16, kb16, msk_c[0].T, msk_c[1].T], axis=1, dtype=np.float32))
        m["x8d"] = np.ascontiguousarray(x8[c * BPC : (c + 1) * BPC])
        in_maps.append(m)

    # A rare per-process DMA race can corrupt a core's staging buffer, which
    # surfaces as NaN/Inf.  Sticky per module load: rebuild after 2 failures.
    skip_gb = bool(
        np.all(shared["gamma"] == 1.0) and np.all(shared["beta"] == 0.0)
    )
    out = None
    for attempt in range(6):
        res = run_bass_kernel_spmd(
            _get_nc(skip_gb), in_maps, core_ids=list(range(NCORES))
        )
        out = np.concatenate([res.results[c]["out"] for c in range(NCORES)], axis=0)
        if np.isfinite(out).all():
            break
        if attempt >= 1:
            _nc_cache[skip_gb] = None
    return out



# revision 68
# speedup vs baseline: 1.1130x; 1.0430x over previous
"""Trainium2 Bass kernel for BertAttention (B=16, S=1024, H=768, 12 heads).

Data-parallel over batch across 8 NeuronCores (2 batch rows per core).

Host side (in kernel()): weights are pre-transposed to lhsT layout,
pre-scaled by 16 and quantized to fp8e4; x is pre-transposed/quantized
likewise.  hs stays f32 for the residual + layernorm path.

Per-core device kernel:
  - Q/K/V projections as fp8e4 DoubleRow matmuls (0.5 cyc/row, 256-deep
    contraction per matmul).
  - scores as fp8e4 DoubleRow matmuls with Q/K in a feature-paired
    [32, 2, tok] SBUF layout produced by an SBUF->SBUF DMA rearrange
    (4 half-heads per 128 partitions, 3 head-pairs along the free axis).
  - exp on ACT engine with the 1/(8*256) scale folded in, writing fp8
    probs directly; the additive mask is applied exactly as a
    multiplicative exp(mask) folded into the V rows and denominator.
  - softmax denominator folded INTO the PV matmul: lhsT = [V_h | em] for
    even heads and [em | V_h] for odd heads, so denominators accumulate on
    the other 64 PSUM partitions for free.  Division via DVE reciprocal +
    multiply with mixed-partition-offset operands (PSUM in0 + SBUF in1).
  - output projection in bf16 + residual add + LayerNorm
    (bn_stats/bn_aggr, Sqrt batched per t8 pair, gamma in bf16).
  - schedule keeps the ACT engine (the bottleneck at ~216us busy)
    saturated: PE emits scores one k-chunk ahead of PV, batch row b1's
    projections ride in b0's attention slack, and output tiles interleave
    into the following attention loop.

Workaround: this container's walrus accepts only ONE sync wait per
instruction; a post-pass splits multi-wait instructions into single-wait
NOPs.
"""

import numpy as np

import concourse.bass as bass
import concourse.mybir as mybir
import concourse.tile as tile

P = 128
H = 768
NH = 12
HD = 64
S = 1024
B = 16
NCORES = 8
BPC = B // NCORES  # 2
IO_T = H // P      # 6
KO_T = S // P      # 8
HP = NH // 2       # 6 head pairs (one per 128-feature block)
WSCALE = 16.0
EXP_SCALE = 1.0 / (8.0 * WSCALE * WSCALE)  # 1/sqrt(64) / (16*16)
LN_EPS = 1e-12

F32 = mybir.dt.float32
BF16 = mybir.dt.bfloat16
FP8 = mybir.dt.float8e4
AF = mybir.ActivationFunctionType
OP = mybir.AluOpType
PM = mybir.MatmulPerfMode


def _split_multi_waits(nc):
    """walrus here rejects >1 sync wait per instruction; hoist extras into
    single-wait NOPs on the same engine immediately before."""
    n = 0
    for blk in nc.m.functions[0].blocks:
        insts = blk.instructions
        new = []
        changed = False
        for inst in insts:
            si = inst.sync_info
            waits = list(si.on_wait) if si and si.on_wait else []
            if len(waits) > 1:
                changed = True
                for k, w in enumerate(waits[:-1]):
                    n += 1
                    new.append(
                        mybir.InstNoOp(
                            name=f"ws-{blk.name}-{inst.name}-{k}",
                            engine=inst.engine,
                            sync_info=mybir.SyncInfo(on_wait=[w], on_update=[]),
                        )
                    )
                inst.sync_info = mybir.SyncInfo(
                    on_wait=[waits[-1]], on_update=list(si.on_update)
                )
            new.append(inst)
        if changed:
            blk.instructions = new
    return n


def _bcast_ap(ap, parts=P):
    return bass.AP(tensor=ap.tensor, offset=ap.offset, ap=[[0, parts]] + list(ap.ap))


def build_bass(skip_gb=False):
    nc = bass.Bass()

    # weights / x arrive pre-transposed, pre-scaled (x16) and pre-quantized
    # to fp8e4 from the host; hs stays f32 for the residual + layernorm path.
    hs = nc.declare_dram_parameter("hs", [BPC, S, H], F32, isOutput=False)
    x8d = nc.declare_dram_parameter("x8d", [BPC, P, IO_T, S], FP8, isOutput=False)
    scon = nc.declare_dram_parameter("scon", [P, 28], F32, isOutput=False)
    w8qk = nc.declare_dram_parameter("w8qk", [P, 2, IO_T, H], FP8, isOutput=False)
    w8v = nc.declare_dram_parameter("w8v", [P, IO_T, H], FP8, isOutput=False)
    wTo = nc.declare_dram_parameter("wTo", [P, 2, 3, H], FP8, isOutput=False)
    vb16d = nc.declare_dram_parameter("vb16d", [H], BF16, isOutput=False)
    obd = nc.declare_dram_parameter("obd", [H], BF16, isOutput=False)
    gamma = nc.declare_dram_parameter("gamma", [H], F32, isOutput=False)
    beta = nc.declare_dram_parameter("beta", [H], F32, isOutput=False)
    out = nc.declare_dram_parameter("out", [BPC, S, H], F32, isOutput=True)

    from contextlib import ExitStack

    with tile.TileContext(nc) as tc:
        with ExitStack() as ctx:
            _build_tile(
                ctx, tc, nc, hs, x8d, scon, w8qk, w8v, wTo,
                vb16d, obd, gamma, beta, out, skip_gb
            )

    _split_multi_waits(nc)
    return nc


def _build_tile(ctx, tc, nc, hs, x8d, scon, w8qk, w8v, wTo,
                vb16d, obd, gamma, beta, out, skip_gb=False):
    dram = ctx.enter_context(tc.tile_pool(name="dram", bufs=1, space="DRAM"))
    consts = ctx.enter_context(tc.tile_pool(name="consts", bufs=1))
    perb = ctx.enter_context(tc.tile_pool(name="perb", bufs=2))
    x8_pool = ctx.enter_context(tc.tile_pool(name="x8", bufs=2))
    pre_pool = ctx.enter_context(tc.tile_pool(name="pre", bufs=2))
    qk8_pool = ctx.enter_context(tc.tile_pool(name="qk8", bufs=4))
    ve_pool = ctx.enter_context(tc.tile_pool(name="ve", bufs=2))
    pt_pool = ctx.enter_context(tc.tile_pool(name="pt", bufs=5))
    rcp_pool = ctx.enter_context(tc.tile_pool(name="rcp", bufs=2))
    ctxT_pool = ctx.enter_context(tc.tile_pool(name="ctxT", bufs=2))
    xres_pool = ctx.enter_context(tc.tile_pool(name="xres", bufs=2))
    s_pool = ctx.enter_context(tc.tile_pool(name="s", bufs=4))
    n_pool = ctx.enter_context(tc.tile_pool(name="n", bufs=2))
    o_pool = ctx.enter_context(tc.tile_pool(name="o", bufs=2))
    ln_pool = ctx.enter_context(tc.tile_pool(name="ln", bufs=4))

    ps_proj = ctx.enter_context(tc.tile_pool(name="psp", bufs=2, space="PSUM"))
    ps_sc = ctx.enter_context(tc.tile_pool(name="pssc", bufs=2, space="PSUM"))
    ps_pv = ctx.enter_context(tc.tile_pool(name="pspv", bufs=1, space="PSUM"))

    # ---------------- constants / weight staging --------------------------
    w8 = {}
    x8s = []
    for b in range(BPC):
        x8s.append(x8_pool.tile([P, IO_T, S], FP8, tag="x8", name=f"x8_{b}"))
    w8qk_sb = consts.tile([P, 2, IO_T, H], FP8, tag="w8qk_sb", name="w8qk_sb")
    w8["q"] = w8qk_sb[:, 0]
    w8["k"] = w8qk_sb[:, 1]
    w8["v"] = consts.tile([P, IO_T, H], FP8, tag="w8_v", name="w8_v")
    wT_o = consts.tile([P, 2, 3, H], FP8, tag="wT_o", name="wT_o")
    # DMA transfers serialize on the (modeled) DMA engine block, so the
    # startup order IS the critical path: masks first (unblocks the em exp),
    # then x8 token-halves, then only the jo0 column slices of wq/wk so the
    # first head-pair's projections start ~4us earlier than a full-w load.
    # single queue, strict order: the DMA block drains jobs in HWDGE-gen
    # completion order, and HWDGE round-robins across queues -- a second
    # queue's bulk load would jump ahead of the critical x8 halves.
    scon_sb = consts.tile([P, 28], F32, tag="scon_sb", name="scon_sb")
    nc.sync.dma_start(out=w8qk_sb[:, :, :, 0:P], in_=w8qk[:, :, :, 0:P])
    nc.sync.dma_start(out=scon_sb, in_=scon[:, :])
    nc.sync.dma_start(out=x8s[0][:, :, 0:512], in_=x8d[0][:, :, 0:512])
    nc.sync.dma_start(out=x8s[0][:, :, 512:S], in_=x8d[0][:, :, 512:S])
    nc.sync.dma_start(out=w8qk_sb[:, :, :, P:H], in_=w8qk[:, :, :, P:H])
    nc.sync.dma_start(out=w8["v"], in_=w8v[:, :, :])

    def stage_rest():
        nc.sync.dma_start(out=x8s[1], in_=x8d[:, :, :, :][1])
        nc.sync.dma_start(out=wT_o, in_=wTo[:, :, :, :])
        nc.sync.dma_start(out=ob_row, in_=obd[:][None, :])
        nc.gpsimd.dma_start(out=gamma_bc, in_=_bcast_ap(gamma[:]))
        nc.gpsimd.dma_start(out=beta_bc, in_=_bcast_ap(beta[:]))

    qb16 = scon_sb[:, 0:IO_T]
    kb16 = scon_sb[:, IO_T : 2 * IO_T]
    vb16_row = consts.tile([1, H], BF16, tag="vb16_row")
    nc.scalar.dma_start(out=vb16_row, in_=vb16d[:][None, :])
    ob_row = consts.tile([1, H], BF16, tag="ob_row")
    gamma_bc = consts.tile([P, H], BF16, tag="gamma_bc")
    beta_bc = consts.tile([P, H], F32, tag="beta_bc")

    eps_sb = consts.tile([P, 1], F32, tag="eps")
    nc.vector.memset(eps_sb, LN_EPS)
    ones_row = consts.tile([1, P], BF16, tag="ones_row")
    nc.vector.memset(ones_row, 1.0)
    ones16 = consts.tile([P, 6, 2, HD], BF16, tag="ones16")
    nc.vector.memset(ones16, WSCALE)

    # ---------------- per-b state ----------------------------------------
    qk_tiles = {}
    em_sbs = [None] * BPC
    Q8s = [None] * BPC
    K8s = [None] * BPC
    VEs = [None] * BPC
    ctxTs = [None] * BPC

    def emit_mask(b):
        mask_sb = scon_sb[:, 12 + KO_T * b : 12 + KO_T * (b + 1)]
        em_sbs[b] = perb.tile([P, KO_T], F32, tag="em", name=f"em_{b}")
        nc.scalar.activation(out=em_sbs[b], in_=mask_sb, func=AF.Exp)

    def emit_qk_proj(b, names=("q", "k"), use_act=False, gs=(0, 1), jos=None,
                     tts=(0, 1), do_mm=True, do_rearr=True):
        """Q/K projections (fp8 DR) -> fp8 pre tiles -> DMA pair-rearrange.

        Emitted g-outer (3-jo group), tensor-inner, so the first head-pairs
        of BOTH Q and K are ready before later groups.  use_act routes the
        PSUM->fp8+bias copies to the ACT engine; default is DVE.  jos
        (absolute jo indices) selects jo-granular emission + rearrange so a
        single head-pair's Q/K can be staged on the startup critical path."""
        x8 = x8s[b]
        cfg = {"q": (qb16, Q8s), "k": (kb16, K8s)}
        tiles = {}
        for wname in names:
            if (b, wname) not in qk_tiles:
                qk_tiles[(b, wname)] = (
                    pre_pool.tile([P, IO_T, S], FP8, tag="pre", name=f"{wname}pre_{b}"),
                    qk8_pool.tile([P, 2, 3, S], FP8, tag="qk8", name=f"{wname}8_{b}"),
                )
            tiles[wname] = qk_tiles[(b, wname)]
            cfg[wname][1][b] = tiles[wname][1]

        def emit_jo(wname, jo):
            bias, _ = cfg[wname]
            pre, paired = tiles[wname]
            for tt in tts:
                ps = ps_proj.tile([P, 512], F32, tag="proj")
                for i2 in range(3):
                    lhsT = w8[wname][:, 2 * i2 : 2 * i2 + 2, jo * P : (jo + 1) * P]
                    nc.tensor.matmul(
                        ps,
                        lhsT=lhsT,
                        rhs=x8[:, 2 * i2 : 2 * i2 + 2, tt * 512 : (tt + 1) * 512],
                        start=(i2 == 0),
                        stop=(i2 == 2),
                        perf_mode=PM.DoubleRow,
                    )
                if use_act is True or (use_act == "tt0" and tt == 0):
                    nc.scalar.activation(
                        out=pre[:, jo, tt * 512 : (tt + 1) * 512],
                        in_=ps,
                        func=AF.Identity,
                        bias=bias[:, jo : jo + 1],
                    )
                else:
                    nc.vector.tensor_scalar_add(
                        out=pre[:, jo, tt * 512 : (tt + 1) * 512],
                        in0=ps,
                        scalar1=bias[:, jo : jo + 1],
                    )

        def rearrange(wname, g, jo_lo, jo_hi):
            # pair-rearrange: head-pair hp -> (g = hp//3, s = hp%3);
            # half-head (hp, A) at partitions 64g..64g+32, (hp, B) at +32.
            # paired[p, i, s, n]: feature 2(p%32)+i of that half-head.
            pre, paired = tiles[wname]
            eng = nc.sync
            for half in range(2):
                src = pre[64 * half : 64 * half + 64, jo_lo:jo_hi, :].rearrange(
                    "(p i) jo n -> p i jo n", i=2
                )
                pb = 64 * g + 32 * half
                for i in range(2):
                    eng.dma_start(
                        out=paired[pb : pb + 32, i, jo_lo - 3 * g : jo_hi - 3 * g, :],
                        in_=src[:, i, :, :],
                    )

        if jos is not None:
            if do_mm:
                for jo in jos:
                    for wname in names:
                        emit_jo(wname, jo)
            if not do_rearr:
                return
            # rearrange contiguous runs (within one g) per tensor
            run = [jos[0], jos[0] + 1]
            runs = [run]
            for jo in jos[1:]:
                if jo == run[1] and jo // 3 == run[0] // 3:
                    run[1] = jo + 1
                else:
                    run = [jo, jo + 1]
                    runs.append(run)
            for wname in names:
                for lo, hi in runs:
                    rearrange(wname, lo // 3, lo, hi)
            return
        for g in gs:
            for wname in names:
                for jo in range(3 * g, 3 * g + 3):
                    emit_jo(wname, jo)
                rearrange(wname, g, 3 * g, 3 * g + 3)

    def emit_v_proj(b, t8s=None, use_act=False):
        """V projection (fp8 DR); write VE = per-head [V|em] / [em|V] fp8.

        use_act routes the PSUM->fp8 em-scaled copies to the ACT engine
        (only sensible pre-stream, when ACT is otherwise idle)."""
        x8 = x8s[b]
        em_sb = em_sbs[b]
        if VEs[b] is None:
            VEs[b] = ve_pool.tile([P, KO_T, NH, P], FP8, tag="VE", name=f"VE_{b}")
        VE = VEs[b]
        for t8 in (t8s if t8s is not None else range(KO_T)):
            # every head's VE row is [V | em]: em columns 64:128
            ve_all = VE[:, t8, :, :].rearrange("p (hh two) d -> p hh two d", two=2)
            nc.vector.tensor_scalar_mul(
                out=ve_all[:, :, :, HD:P],
                in0=ones16,
                scalar1=em_sb[:, t8 : t8 + 1],
            )
            for jh in range(2):
                ps = ps_proj.tile([P, 512], F32, tag="proj")
                for i2 in range(3):
                    lhsT = x8[:, 2 * i2 : 2 * i2 + 2, t8 * P : (t8 + 1) * P]
                    nc.tensor.matmul(
                        ps[:, 0:384],
                        lhsT=lhsT,
                        rhs=w8["v"][:, 2 * i2 : 2 * i2 + 2,
                                    jh * 384 : (jh + 1) * 384],
                        start=(i2 == 0),
                        stop=False,
                        perf_mode=PM.DoubleRow,
                    )
                nc.tensor.matmul(
                    ps[:, 0:384],
                    lhsT=ones_row,
                    rhs=vb16_row[:, jh * 384 : (jh + 1) * 384],
                    start=False,
                    stop=True,
                )
                # heads 6jh..6jh+5 live in psum cols (h-6jh)*64; every
                # head's V block goes to VE cols 0:64 (em lives in 64:128)
                ps_v = ps[:, 0:384].rearrange(
                    "p (hh two d) -> p hh two d", two=2, d=HD
                )
                ve_jh = VE[:, t8, 6 * jh : 6 * jh + 6, :].rearrange(
                    "p (hh two) d -> p hh two d", two=2
                )
                if use_act:
                    nc.scalar.activation(
                        out=ve_jh[:, :, :, 0:HD], in_=ps_v,
                        func=AF.Copy, scale=em_sb[:, t8 : t8 + 1],
                    )
                else:
                    nc.vector.tensor_scalar_mul(
                        out=ve_jh[:, :, :, 0:HD],
                        in0=ps_v,
                        scalar1=em_sb[:, t8 : t8 + 1],
                    )

    attn_pts = {}

    def emit_attn(b, hp, qt, cbs=None, defer_pv=False, direct=False, split=False,
                  kcs=None, div_halves=False):
        """scores (fp8 DR) -> exp -> PV(+denominator) -> divide, one q-chunk.

        defer_pv emits all scores before any PV matmul: used for the first
        unit(s), where PV would stall on VE writes at the PE queue head and
        block later score matmuls (head-of-line) from feeding the ACT exp
        stream.  direct reads scores straight from the un-paired pre tiles
        (non-DR fp8, 2x PE cost) to skip the pair-rearrange on the startup
        critical path.  split=True emits only scores+exp and returns a
        finisher that emits PV+divide (call it later, once VE is staged)."""
        Q8, K8 = Q8s[b], K8s[b]
        if ctxTs[b] is None:
            ctxTs[b] = ctxT_pool.tile([P, 2, 3, S], FP8, tag="ctxT", name=f"ctxT_{b}")
        ctxT = ctxTs[b]
        qsl0 = qt * 512
        if (b, hp, qt) in attn_pts:
            pt = attn_pts[(b, hp, qt)]
        else:
            pt = pt_pool.tile([P, 2, KO_T, 512], FP8, tag="pt",
                              name=f"pt_{b}_{hp}_{qt}")
            attn_pts[(b, hp, qt)] = pt
        g, sslot = hp // 3, hp % 3

        def emit_scores(kc):
            for dst, pbase in ((0, 64 * g), (1, 64 * g + 32)):
                sc = ps_sc.tile([P, 2, 512], F32, tag="sc")
                for k2 in range(2):
                    ko = kc * 2 + k2
                    if direct:
                        preq = qk_tiles[(b, "q")][0]
                        prek = qk_tiles[(b, "k")][0]
                        db = 64 * dst
                        nc.tensor.matmul(
                            sc[:, k2, :],
                            lhsT=prek[db : db + 64, hp, ko * P : (ko + 1) * P],
                            rhs=preq[db : db + 64, hp, qsl0 : qsl0 + 512],
                            start=True,
                            stop=True,
                            tile_position=(db, 0),
                        )
                        continue
                    lhsT = K8[pbase : pbase + 32, :, sslot, ko * P : (ko + 1) * P]
                    nc.tensor.matmul(
                        sc[:, k2, :],
                        lhsT=lhsT,
                        rhs=Q8[pbase : pbase + 32, :, sslot, qsl0 : qsl0 + 512],
                        start=True,
                        stop=True,
                        perf_mode=PM.DoubleRow,
                        tile_position=(pbase, 0),
                    )
                nc.scalar.activation(
                    out=pt[:, dst, kc * 2 : kc * 2 + 2, :],
                    in_=sc,
                    func=AF.Exp,
                    scale=EXP_SCALE,
                )

        ctxp = [None, None]

        def alloc_pv():
            ctxp[0] = ps_pv.tile([P, 512], F32, tag="pvA", name=f"pvA_{b}_{hp}_{qt}")
            ctxp[1] = ps_pv.tile([P, 512], F32, tag="pvB", name=f"pvB_{b}_{hp}_{qt}")

        def emit_pv(kc):
            # DoubleRow: contract 256 keys (ko pair 2kc, 2kc+1) per matmul
            VE = VEs[b]
            nc.tensor.matmul(
                ctxp[0],
                lhsT=VE[:, 2 * kc : 2 * kc + 2, 2 * hp, :],
                rhs=pt[:, 0, 2 * kc : 2 * kc + 2, :],
                start=(kc == 0),
                stop=(kc == KO_T // 2 - 1),
                perf_mode=PM.DoubleRow,
            )
            nc.tensor.matmul(
                ctxp[1],
                lhsT=VE[:, 2 * kc : 2 * kc + 2, 2 * hp + 1, :],
                rhs=pt[:, 1, 2 * kc : 2 * kc + 2, :],
                start=(kc == 0),
                stop=(kc == KO_T // 2 - 1),
                perf_mode=PM.DoubleRow,
            )

        def emit_div():
            # ctxp[d]: rows 0:64 = 16*ctx, rows 64:128 = 16*den (both dsts).
            # Partition-offset operand only ever on in1 (the proven pattern).
            # div_halves splits by query columns so the first half's ctxT
            # (feeding the first tail output tiles) lands earlier.
            rcpT = rcp_pool.tile([P, 2, 512], F32, tag="rcpT")
            cols = ((0, 256), (256, 512)) if div_halves else ((0, 512),)
            for c0, c1 in cols:
                nc.vector.reciprocal(
                    out=rcpT[HD:P, 0, c0:c1], in_=ctxp[0][HD:P, c0:c1])
                nc.vector.reciprocal(
                    out=rcpT[HD:P, 1, c0:c1], in_=ctxp[1][HD:P, c0:c1])
                nc.vector.tensor_tensor(
                    out=ctxT[0:HD, hp % 2, hp // 2, qsl0 + c0 : qsl0 + c1],
                    in0=ctxp[0][0:HD, c0:c1],
                    in1=rcpT[HD:P, 0, c0:c1],
                    op=OP.mult,
                )
                nc.vector.tensor_tensor(
                    out=ctxT[HD:P, hp % 2, hp // 2, qsl0 + c0 : qsl0 + c1],
                    in0=rcpT[HD:P, 1, c0:c1],
                    in1=ctxp[1][0:HD, c0:c1],
                    op=OP.mult,
                )

        if split:
            for kc in (kcs if kcs is not None else range(KO_T // 2)):
                emit_scores(kc)

            def finish():
                alloc_pv()
                for kc in range(KO_T // 2):
                    emit_pv(kc)
                emit_div()

            return finish

        alloc_pv()
        # PE order: sc0 sc1 pv0 sc2 pv1 sc3 pv2 pv3 -- keeps the exp stream
        # fed one chunk ahead so ACT never waits on PV matmuls.  cbs inject
        # foreign PE work (output-projection halves) into the slack.
        for kc in range(KO_T // 2):
            emit_scores(kc)
            if cbs and kc in cbs:
                cbs[kc]()
            if not defer_pv and kc >= 1:
                emit_pv(kc - 1)
        if defer_pv:
            for kc in range(KO_T // 2):
                emit_pv(kc)
        else:
            emit_pv(KO_T // 2 - 1)
        emit_div()

    # ---- output projection + residual + layernorm ------------------------
    mv_alls = [None] * BPC
    rstds = [None] * BPC
    s_tiless = [[], []]

    def emit_out_jh(b, t8, jh, state, alt_ps=False):
        # alt_ps parks the o-proj psum in the (idle-at-tail) score banks as
        # one [P, 2, 512] tile per t8 so all tail tiles project concurrently
        # and the residual merges into a single TT.
        ctxT = ctxTs[b]
        if jh == 0:
            state["xres"] = xres_pool.tile([P, H], F32, tag="xres", name=f"xres_{b}_{t8}")
            nc.sync.dma_start(
                out=state["xres"], in_=hs[b, t8 * P : (t8 + 1) * P, :]
            )
            state["s_t"] = s_pool.tile([P, H], F32, tag="s", name=f"s_{b}_{t8}")
            if alt_ps:
                state["ps2"] = ps_sc.tile([P, 2, 512], F32, tag="sc",
                                          name=f"ops_{b}_{t8}")
        xres = state["xres"]
        s_t = state["s_t"]
        if alt_ps:
            pso = state["ps2"][:, jh, 0:384]
        else:
            ps = ps_proj.tile([P, 512], F32, tag="proj")
            pso = ps[:, 0:384]
        for g in range(3):
            nc.tensor.matmul(
                pso,
                lhsT=ctxT[:, :, g, t8 * P : (t8 + 1) * P],
                rhs=wT_o[:, :, g, jh * 384 : (jh + 1) * 384],
                start=(g == 0),
                stop=False,
                perf_mode=PM.DoubleRow,
            )
        nc.tensor.matmul(
            pso,
            lhsT=ones_row,
            rhs=ob_row[:, jh * 384 : (jh + 1) * 384],
            start=False,
            stop=True,
        )
        if alt_ps:
            if jh == 1:
                nc.vector.tensor_tensor(
                    out=s_t.rearrange("p (j c) -> p j c", j=2),
                    in0=state["ps2"][:, :, 0:384],
                    in1=xres.rearrange("p (j c) -> p j c", j=2),
                    op=OP.add,
                )
            return
        nc.vector.tensor_tensor(
            out=s_t[:, jh * 384 : (jh + 1) * 384],
            in0=ps[:, 0:384],
            in1=xres[:, jh * 384 : (jh + 1) * 384],
            op=OP.add,
        )

    def emit_out(b, t8, pool_gb=False, state=None, rbatch=2, split_out=False):
        if mv_alls[b] is None:
            mv_alls[b] = ln_pool.tile([P, KO_T, 2], F32, tag="mv", name=f"mv_{b}")
            rstds[b] = ln_pool.tile([P, KO_T], F32, tag="rstd", name=f"rstd_{b}")
        mv_all = mv_alls[b]
        rstd = rstds[b]
        s_tiles = s_tiless[b]

        if state is None:
            state = {}
            emit_out_jh(b, t8, 0, state, alt_ps=split_out)
            emit_out_jh(b, t8, 1, state, alt_ps=split_out)
        s_t = state["s_t"]
        stats = ln_pool.tile([P, 2, 6], F32, tag="stats")
        for sg in range(2):
            nc.vector.bn_stats(
                out=stats[:, sg, :], in_=s_t[:, sg * 384 : (sg + 1) * 384]
            )
        nc.vector.bn_aggr(out=mv_all[:, t8, :], in_=stats)
        s_tiles.append(s_t)

        if t8 % rbatch == rbatch - 1:
            h0 = t8 - (rbatch - 1)
            nc.scalar.activation(
                out=rstd[:, h0 : t8 + 1],
                in_=mv_all[:, h0 : t8 + 1, 1],
                func=AF.Sqrt,
                bias=eps_sb,
                scale=1.0,
            )
            nc.vector.reciprocal(out=rstd[:, h0 : t8 + 1], in_=rstd[:, h0 : t8 + 1])
            for u8 in range(h0, t8 + 1):
                # split_out halves the normalize + store so the final DMA
                # overlaps the second half's normalize (tail latency only)
                halves = ((0, 384), (384, H)) if split_out else ((0, H),)
                n_t = n_pool.tile([P, H], F32 if skip_gb else BF16, tag="n")
                for hi, (c0, c1) in enumerate(halves):
                    eng = (nc.vector if (split_out and hi == 1) else
                           (nc.gpsimd if pool_gb else nc.vector))
                    eng.tensor_scalar(
                        out=n_t[:, c0:c1],
                        in0=s_tiles[u8][:, c0:c1],
                        scalar1=mv_all[:, u8, 0:1],
                        scalar2=rstd[:, u8 : u8 + 1],
                        op0=OP.subtract,
                        op1=OP.mult,
                    )
                    if skip_gb:
                        # gamma==1, beta==0: the affine step is the identity
                        nc.sync.dma_start(
                            out=out[b, u8 * P : (u8 + 1) * P, c0:c1],
                            in_=n_t[:, c0:c1],
                        )
                        continue
                    g_t = o_pool.tile([P, H], BF16, tag="g")
                    eng.tensor_tensor(out=g_t[:, c0:c1], in0=n_t[:, c0:c1],
                                      in1=gamma_bc[:, c0:c1], op=OP.mult)
                    o_t = o_pool.tile([P, H], F32, tag="o")
                    eng.tensor_tensor(out=o_t[:, c0:c1], in0=g_t[:, c0:c1],
                                      in1=beta_bc[:, c0:c1], op=OP.add)
                    nc.sync.dma_start(
                        out=out[b, u8 * P : (u8 + 1) * P, c0:c1],
                        in_=o_t[:, c0:c1],
                    )

    # ---------------- schedule -------------------------------------------
    # Startup: stage q/k jo-granular so hp0's attention starts after jo0;
    # v-proj interleaves so the DVE does jo-adds and VE writes just-in-time.
    # hp0-qt0 runs "direct" off the un-paired jo0 pre tiles the moment the
    # jo0 tt0 projections land (kc0/kc1 only need keys 0:512 and the qt0
    # queries 0:512 = tt0); tt1 + kc2/kc3 follow.  PV+divide deferred until
    # VE is staged.
    emit_qk_proj(0, jos=(0,), use_act=True, tts=(0,), do_rearr=False)
    emit_attn(0, 0, 0, direct=True, split=True, kcs=(0, 1))
    emit_qk_proj(0, jos=(0,), tts=(1,), do_rearr=False)
    finish_hp0 = emit_attn(0, 0, 0, direct=True, split=True, kcs=(2, 3))
    emit_mask(0)
    emit_mask(1)
    emit_qk_proj(0, jos=(1,))
    emit_qk_proj(0, jos=(2,))
    emit_v_proj(0, t8s=(0, 1))
    emit_qk_proj(0, jos=(3,))
    emit_v_proj(0, t8s=(2, 3))
    emit_qk_proj(0, jos=(4,))
    emit_qk_proj(0, jos=(5,))
    emit_v_proj(0, t8s=(4, 5))
    emit_v_proj(0, t8s=(6, 7))
    emit_qk_proj(0, jos=(0,), do_mm=False)
    stage_rest()

    # b0 attention qt0, unit-pipelined: unit N's scores are emitted before
    # unit N-1's PV+divide so a VE-staging stall at the PE queue head never
    # starves the exp stream.  b1 projections slot into the PE slack.
    prev_fin = finish_hp0
    for hp in range(1, HP):
        fin = emit_attn(0, hp, 0, split=True)
        prev_fin()
        prev_fin = fin
        if hp == 3:
            emit_qk_proj(1, names=("q",), gs=(0,))
        if hp == 4:
            emit_qk_proj(1, names=("q",), gs=(1,))
        if hp == 5:
            emit_qk_proj(1, names=("k",), gs=(0,))
    prev_fin()
    def attn_with_out(ab, hp, qt, ob, t8, pool_gb=False):
        emit_attn(ab, hp, qt)
        emit_out(ob, t8, pool_gb=pool_gb)

    for hp in range(HP):
        if 1 <= hp <= 4:
            attn_with_out(0, hp, 1, 0, hp - 1, pool_gb=True)
        else:
            emit_attn(0, hp, 1)
        if hp == 0:
            emit_qk_proj(1, names=("k",), gs=(1,))
        if hp <= 3:
            emit_v_proj(1, t8s=(2 * hp, 2 * hp + 1))
    for hp in range(HP):
        if hp <= 3:
            attn_with_out(1, hp, 0, 0, 4 + hp, pool_gb=True)
        else:
            emit_attn(1, hp, 0)
    for hp in range(HP):
        if hp <= 3:
            attn_with_out(1, hp, 1, 1, hp, pool_gb=True)
        else:
            emit_attn(1, hp, 1, div_halves=(hp == HP - 1))
    for t8 in range(4, KO_T):
        emit_out(1, t8, rbatch=1, pool_gb=True, split_out=True)


_nc_cache = {}


def _get_nc(skip_gb=True):
    if _nc_cache.get(skip_gb) is None:
        _nc_cache[skip_gb] = build_bass(skip_gb)
    return _nc_cache[skip_gb]


def kernel(**inputs):
    import ml_dtypes
    from concourse.bass_utils import run_bass_kernel_spmd

    E4M3 = ml_dtypes.float8_e4m3
    BF = ml_dtypes.bfloat16

    def wt8(w):
        # w [H, H] (torch Linear weight): lhsT layout [128, IO_T, H] of 16*w^T
        wt = np.asarray(w, np.float32).T.reshape(IO_T, P, H).transpose(1, 0, 2)
        return np.ascontiguousarray((wt * WSCALE).astype(E4M3))

    def _wto_dr(ow):
        p = np.arange(P)[:, None, None]
        i = np.arange(2)[None, :, None]
        g = np.arange(3)[None, None, :]
        f = 64 * (4 * g + 2 * i + (p >= 64)) + (p % 64)  # [128, 2, 3]
        return np.ascontiguousarray((WSCALE * ow.T.astype(np.float32))[f, :].astype(E4M3))

    hs = np.asarray(inputs["hidden_states"], np.float32)
    mask = np.asarray(inputs["attention_mask"], np.float32).reshape(B, S)
    # x^T fp8: [B, 128, IO_T, S]
    x8 = np.ascontiguousarray(
        hs.transpose(0, 2, 1).reshape(B, IO_T, P, S).transpose(0, 2, 1, 3).astype(E4M3)
    )
    shared = {
        "w8qk": np.ascontiguousarray(
            np.stack([wt8(inputs["qw"]), wt8(inputs["kw"])], axis=1)
        ),
        "w8v": wt8(inputs["vw"]),
        # o-proj lhsT pairing for DoubleRow: contraction row (p, i) in group g
        # is ctx feature f = 64*(4g+2i+(p>=64)) + p%64 (matches ctxT layout
        # [p, i=hp%2, g=hp//2] with head A on partitions 0:64, B on 64:128).
        "wTo": _wto_dr(np.asarray(inputs["ow"], np.float32)),

        "vb16d": np.ascontiguousarray(
            (np.asarray(inputs["vb"], np.float32) * WSCALE).astype(BF)
        ),
        "obd": np.ascontiguousarray(
            (np.asarray(inputs["ob"], np.float32) * WSCALE).astype(BF)
        ),
        "gamma": np.ascontiguousarray(np.asarray(inputs["gamma"], np.float32)),
        "beta": np.ascontiguousarray(np.asarray(inputs["beta"], np.float32)),
    }
    # residual staged pre-scaled by 16 so the fp8 o-proj's 16x weight scale
    # cancels inside the (scale-invariant) layernorm: s = 16*(dense+ob+hs)
    hs16 = hs * WSCALE
    qb16 = (np.asarray(inputs["qb"], np.float32) * WSCALE).reshape(IO_T, P).T
    kb16 = (np.asarray(inputs["kb"], np.float32) * WSCALE).reshape(IO_T, P).T
    in_maps = []
    for c in range(NCORES):
        m = dict(shared)
        m["hs"] = np.ascontiguousarray(hs16[c * BPC : (c + 1) * BPC])
        msk_c = mask[c * BPC : (c + 1) * BPC].reshape(BPC, KO_T, P)
        m["scon"] = np.ascontiguousarray(np.concatenate(
            [qb16, kb16, msk_c[0].T, msk_c[1].T], axis=1, dtype=np.float32))
        m["x8d"] = np.ascontiguousarray(x8[c * BPC : (c + 1) * BPC])
        in_maps.append(m)

    # A rare per-process DMA race can corrupt a core's staging buffer, which
    # surfaces as NaN/Inf.  Sticky per module load: rebuild after 2 failures.
    skip_gb = bool(
        np.all(shared["gamma"] == 1.0) and np.all(shared["beta"] == 0.0)
    )
    out = None
    for attempt in range(6):
        res = run_bass_kernel_spmd(
            _get_nc(skip_gb), in_maps, core_ids=list(range(NCORES))
        )
        out = np.concatenate([res.results[c]["out"] for c in range(NCORES)], axis=0)
        if np.isfinite(out).all():
            break
        if attempt >= 1:
            _nc_cache[skip_gb] = None
    return out

